# revision 43
# baseline (speedup 1.0000x reference)
"""Trainium2 Bass kernel for the 1D advection stencil (slope-limited flux).

Math (axis=-1, L = N + 4 ghost cells, th = 2.0):
    flux = rho * v
    d[i]  = flux[i+1] - flux[i]
    hs[i] = minmod3(d[i], (d[i]+d[i+1])/4, d[i+1])        # == 0.5*minmod3(c0,c1,c2)
    p[i]  = flux[i+1] - hs[i];  q[i] = flux[i+1] + hs[i]
    pm[i] = (v[i+1] < 0) * p[i];  qm[i] = (v[i+1] > 0) * q[i]
    pm[L-3] = 0; qm[0] = 0
    fn[j]  = pm[j+1] + qm[j]
    out[i] = fn[i] - fn[i+1]
minmod3(a,b,c) = max(min3, min(max3, 0)).

SHIPPED: build_module_v7 (see its docstring) -- clamp-form minmod,
13 tensor_tensor ops/chunk, shifted views instead of materialized shifted
copies, HWDGE-only so the graded module also runs inside tc.For_i for the
device-time measurement.  Older builders (v3/v4/v5/v6) are kept for
reference/ablation only.

Implementation notes (measured/derived on this container):
  * DVE rates: tensor_tensor fp32 = 1x, bf16 = 2x (2x_1p, needs 4B-aligned
    starts: odd bf16 element offsets degrade); tensor_scalar bf16 = 4x;
    scalar_tensor_tensor = 1x ALWAYS (no fast uops -> avoided entirely).
  * rel-err budget is 2e-2; full bf16 pipeline measures ~3.8e-3.
  * ACT (scalar engine, 1x @1.2GHz) does dtype conversions, the
    even-aligned shifted copy of f, and the v-sign masks off the DVE
    critical path. ACT bias/scale are per-partition scalars only.
  * Custom fused DVE ops do NOT compile here (walrus codegen rejects
    InstCustomDveAnt: "ISA wrong length") -- verified.
  * Pool/GpSimd shares an SBUF port with DVE (exclusive lock) -- unused.

Sharding: pure data-parallel over the leading batch axis B=16 -> 2 slabs
per core on 8 cores.  No halo exchange needed.
"""

import numpy as np

import concourse.bass as bass
import concourse.mybir as mybir
from concourse.mybir import AluOpType
from concourse.tile import TileContext
from concourse.bass_utils import run_bass_kernel_spmd

# Problem shape (hardcoded; kernel.py must be self-contained).
B, M, L = 16, 256, 8192
NCORES = 8
BP = B // NCORES            # 2 batch slabs per core
ROWS = BP * M               # 512 rows per core
RT = ROWS // 128            # 4 partition tiles of 128 rows
OUT_L = L - 4               # 8188
F32 = mybir.dt.float32
BF16 = mybir.dt.bfloat16
COPY = mybir.ActivationFunctionType.Copy


def _split_multi_waits(nc):
    """Walrus in this environment rejects instructions carrying more than
    one sync wait ("Too many sync wait commands").  Tile freely attaches
    several.  Split: for an instruction with k>1 waits, emit k-1 engine
    NoOps (one wait each) immediately before it, leaving one wait on the
    instruction itself."""
    import copy
    import concourse.mybir as mybir

    counter = [0]

    def mk_nop(engine, wait):
        counter[0] += 1
        return mybir.InstNoOp(
            name=f"waitsplit-{counter[0]}",
            engine=engine,
            ins=[],
            outs=[],
            sync_info=mybir.SyncInfo(on_wait=[wait], on_update=[]),
        )

    m = nc.m
    new_module = copy.replace(m, functions=[])
    for function in m.functions:
        new_function = copy.replace(function, blocks=[])
        new_function.set_allocations_from_list(function.allocations)
        for block in function.blocks:
            new_insts = []
            for inst in block.instructions:
                si = inst.sync_info
                waits = list(si.on_wait) if (si and si.on_wait) else []
                if len(waits) > 1:
                    for w in waits[:-1]:
                        new_insts.append(mk_nop(inst.engine, w))
                    inst.sync_info = mybir.SyncInfo(
                        on_wait=[waits[-1]], on_update=list(si.on_update)
                    )
                new_insts.append(inst)
            new_function.blocks.append(
                copy.replace(block, instructions=new_insts)
            )
        new_module.functions.append(new_function)
    nc.m = new_module


def _emit_v4_chunk(nc, wk, rho, vin, out, r0, c0, C, CMAX, SMAX,
                   act_pm_sh=False, act_fn_sh=False):
    """v4 chunk body: casting SWDGE DMAs for all dtype conversions (no
    fp32 staging tiles, no ACT conversion passes) + the negated minmod
    chain so the unary steps (s4 scale, relu(-hi)) run on ACT.

    Negation bookkeeping: d' = -d, d1' = -d1, s' = -s, u' = -u, w' = -w,
    nlo = -lo, nhi = -hi, nhi0 = relu(nhi) = -min(hi, 0), hs' = -hs.
    Then p = f1 + hs' (add) and q = f1 - hs' (subtract)."""
    SIGN = mybir.ActivationFunctionType.Sign
    RELU = mybir.ActivationFunctionType.Relu
    S = C + 4
    W2 = C + 2

    # Casting loads (SWDGE): DRAM fp32 -> SBUF bf16.
    rho_b = wk.tile([128, SMAX], BF16, tag="rho_b")
    nc.gpsimd.dma_start(rho_b[:, 0:S], rho[r0:r0 + 128, c0:c0 + S])
    v_b = wk.tile([128, SMAX], BF16, tag="v_b")
    nc.gpsimd.dma_start(v_b[:, 0:S], vin[r0:r0 + 128, c0:c0 + S])

    # ACT masks from v_b: sgn = sign(v1); mneg = relu(-sgn) = (v1<0);
    # mpos = relu(sgn) = (v1>0) in-place over sgn.
    sgn = wk.tile([128, CMAX + 2], BF16, tag="sgn")
    nc.scalar.activation(sgn[:, 0:W2], v_b[:, 1:C + 3], SIGN)
    mneg = wk.tile([128, CMAX + 2], BF16, tag="mneg")
    nc.scalar.activation(mneg[:, 0:W2], sgn[:, 0:W2], RELU, scale=-1.0)
    mpos = sgn
    nc.scalar.activation(mpos[:, 0:W2], sgn[:, 0:W2], RELU)

    # DVE: f = rho*v
    f = wk.tile([128, SMAX], BF16, tag="f")
    nc.vector.tensor_tensor(f[:, 0:S], rho_b[:, 0:S], v_b[:, 0:S],
                            AluOpType.mult)
    # s' = f[i] - f[i+2] = -(d0+d1) while ACT copies f_sh
    s = wk.tile([128, CMAX + 2], BF16, tag="s")
    nc.vector.tensor_tensor(s[:, 0:W2], f[:, 0:W2], f[:, 2:C + 4],
                            AluOpType.subtract)
    # ACT: even-aligned shifted copy f_sh[k] = f[k+1]
    f_sh = wk.tile([128, SMAX - 1], BF16, tag="f_sh")
    nc.scalar.activation(f_sh[:, 0:S - 1], f[:, 1:S], COPY)
    # ACT: s4' = 0.25 * s'
    nc.scalar.activation(s[:, 0:W2], s[:, 0:W2], COPY, scale=0.25)
    # d' = f - f_sh;  d1' = f_sh - f[2:]
    d = wk.tile([128, CMAX + 2], BF16, tag="d")
    nc.vector.tensor_tensor(d[:, 0:W2], f[:, 0:W2], f_sh[:, 0:W2],
                            AluOpType.subtract)
    d1 = wk.tile([128, CMAX + 2], BF16, tag="d1")
    nc.vector.tensor_tensor(d1[:, 0:W2], f_sh[:, 0:W2], f[:, 2:C + 4],
                            AluOpType.subtract)
    # u' = max(d', d1') = -u;  w' = min(d', d1') = -w
    u = wk.tile([128, CMAX + 2], BF16, tag="u")
    nc.vector.tensor_tensor(u[:, 0:W2], d[:, 0:W2], d1[:, 0:W2],
                            AluOpType.max)
    w = wk.tile([128, CMAX + 2], BF16, tag="w")
    nc.vector.tensor_tensor(w[:, 0:W2], d[:, 0:W2], d1[:, 0:W2],
                            AluOpType.min)
    # nlo = max(u', s4') = -lo;  nhi = min(w', s4') = -hi
    nc.vector.tensor_tensor(u[:, 0:W2], u[:, 0:W2], s[:, 0:W2],
                            AluOpType.max)
    nc.vector.tensor_tensor(w[:, 0:W2], w[:, 0:W2], s[:, 0:W2],
                            AluOpType.min)
    # ACT: nhi0 = relu(nhi) = -min(hi,0)   (into s; s4' is dead)
    nc.scalar.activation(s[:, 0:W2], w[:, 0:W2], RELU)
    # hs' = min(nlo, nhi0) = -hs   (into d; d' is dead)
    hs = d
    nc.vector.tensor_tensor(hs[:, 0:W2], u[:, 0:W2], s[:, 0:W2],
                            AluOpType.min)
    # p = f1 - hs = f_sh + hs';  q = f1 + hs = f_sh - hs'
    p = wk.tile([128, CMAX + 2], BF16, tag="p")
    nc.vector.tensor_tensor(p[:, 0:W2], f_sh[:, 0:W2], hs[:, 0:W2],
                            AluOpType.add)
    q = wk.tile([128, CMAX + 2], BF16, tag="q")
    nc.vector.tensor_tensor(q[:, 0:W2], f_sh[:, 0:W2], hs[:, 0:W2],
                            AluOpType.subtract)
    # pm = p*mneg;  qm = q*mpos
    nc.vector.tensor_tensor(p[:, 0:W2], p[:, 0:W2], mneg[:, 0:W2],
                            AluOpType.mult)
    nc.vector.tensor_tensor(q[:, 0:W2], q[:, 0:W2], mpos[:, 0:W2],
                            AluOpType.mult)
    if c0 == 0:
        nc.vector.memset(q[:, 0:1], 0.0)
    if c0 + C == OUT_L:
        nc.vector.memset(p[:, C + 1:C + 2], 0.0)
    # fn = pm[1:] + qm[:-1]; out_b = fn[:-1] - fn[1:]
    if act_pm_sh:
        # even-aligned shifted copy of pm on ACT
        pm_sh = wk.tile([128, CMAX + 1], BF16, tag="pm_sh")
        nc.scalar.activation(pm_sh[:, 0:C + 1], p[:, 1:C + 2], COPY)
        fn = u
        nc.vector.tensor_tensor(fn[:, 0:C + 1], pm_sh[:, 0:C + 1],
                                q[:, 0:C + 1], AluOpType.add)
    else:
        fn = u
        nc.vector.tensor_tensor(fn[:, 0:C + 1], p[:, 1:C + 2],
                                q[:, 0:C + 1], AluOpType.add)
    out_b = w
    if act_fn_sh:
        fn_sh = wk.tile([128, CMAX], BF16, tag="fn_sh")
        nc.scalar.activation(fn_sh[:, 0:C], fn[:, 1:C + 1], COPY)
        nc.vector.tensor_tensor(out_b[:, 0:C], fn[:, 0:C], fn_sh[:, 0:C],
                                AluOpType.subtract)
    else:
        nc.vector.tensor_tensor(out_b[:, 0:C], fn[:, 0:C], fn[:, 1:C + 1],
                                AluOpType.subtract)
    # Casting store (SWDGE): SBUF bf16 -> DRAM fp32.
    nc.gpsimd.dma_start(out[r0:r0 + 128, c0:c0 + C], out_b[:, 0:C])


def build_module(repeat=1, variant="v4", chunk=2730, wk_bufs=2,
                 act_pm_sh=False, act_fn_sh=False):
    """repeat>1 wraps the whole body in a device-side For_i loop --
    benchmark-only, so device time dominates the axon tunnel overhead.

    variant: "v3" (default), "dma" (transfers only -- roofline probe)."""
    import contextlib
    nc = bass.Bass()
    rho = nc.dram_tensor("rho", [ROWS, L], F32, kind="ExternalInput")
    vin = nc.dram_tensor("v", [ROWS, L], F32, kind="ExternalInput")
    out = nc.dram_tensor("out", [ROWS, OUT_L], F32, kind="ExternalOutput")

    SIGN = mybir.ActivationFunctionType.Sign
    RELU = mybir.ActivationFunctionType.Relu

    # All-even chunk sizes covering OUT_L (odd starts degrade bf16 packing,
    # odd lengths break the TS 4x even-dim requirement).
    n_chunks = max(1, round(OUT_L / chunk))
    base = (OUT_L // n_chunks) & ~1
    sizes = [base] * (n_chunks - 1) + [OUT_L - base * (n_chunks - 1)]
    assert all(sz % 2 == 0 for sz in sizes) and sum(sizes) == OUT_L, sizes
    CMAX = max(sizes)
    SMAX = CMAX + 4

    with TileContext(nc) as tc:
        with (
            tc.tile_pool(name="io", bufs=2) as io,
            tc.tile_pool(name="wk", bufs=wk_bufs) as wk,
            (tc.For_i(0, repeat, 1) if repeat > 1 else contextlib.nullcontext()),
        ):
            for rt in range(RT):
                r0 = rt * 128
                c0 = 0
                for C in sizes:
                    S = C + 4
                    if variant == "v4":
                        _emit_v4_chunk(nc, wk, rho, vin, out, r0, c0, C,
                                       CMAX, SMAX,
                                       act_pm_sh=act_pm_sh,
                                       act_fn_sh=act_fn_sh)
                        c0 += C
                        continue
                    rho_t = io.tile([128, SMAX], F32, tag="rho")
                    nc.sync.dma_start(
                        rho_t[:, 0:S], rho[r0:r0 + 128, c0:c0 + S]
                    )
                    v_t = io.tile([128, SMAX], F32, tag="v")
                    nc.sync.dma_start(
                        v_t[:, 0:S], vin[r0:r0 + 128, c0:c0 + S]
                    )
                    if variant == "dma":
                        out_t = io.tile([128, CMAX], F32, tag="out")
                        nc.vector.tensor_tensor(
                            out_t[:, 0:1], rho_t[:, 0:1], v_t[:, 0:1],
                            AluOpType.mult,
                        )
                        nc.sync.dma_start(
                            out[r0:r0 + 128, c0:c0 + C], out_t[:, 0:C]
                        )
                        c0 += C
                        continue
                    if variant == "v4":
                        _emit_v4_chunk(nc, wk, rho, vin, out, r0, c0, C,
                                       CMAX, SMAX)
                        c0 += C
                        continue

                    # --- ACT: conversions + masks (off-DVE) -------------
                    rho_b = wk.tile([128, SMAX], BF16, tag="rho_b")
                    nc.scalar.activation(rho_b[:, 0:S], rho_t[:, 0:S], COPY)
                    v_b = wk.tile([128, SMAX], BF16, tag="v_b")
                    nc.scalar.activation(v_b[:, 0:S], v_t[:, 0:S], COPY)
                    assert variant == "v3", variant
                    # sgn = sign(v1); masks mpos = relu(sgn) = (v1>0),
                    # mneg = relu(-sgn) = (v1<0).  All even-aligned.
                    sgn = wk.tile([128, CMAX + 2], BF16, tag="sgn")
                    nc.scalar.activation(sgn[:, 0:C + 2], v_t[:, 1:C + 3], SIGN)
                    # mneg = relu(-sgn) first, then mpos = relu(sgn)
                    # in-place over sgn (saves a tile tag).
                    mneg = wk.tile([128, CMAX + 2], BF16, tag="mneg")
                    nc.scalar.activation(
                        mneg[:, 0:C + 2], sgn[:, 0:C + 2], RELU, scale=-1.0
                    )
                    mpos = sgn
                    nc.scalar.activation(
                        mpos[:, 0:C + 2], sgn[:, 0:C + 2], RELU
                    )

                    # --- DVE bf16 pipeline ------------------------------
                    # f = rho*v  [S]
                    f = wk.tile([128, SMAX], BF16, tag="f")
                    nc.vector.tensor_tensor(
                        f[:, 0:S], rho_b[:, 0:S], v_b[:, 0:S], AluOpType.mult
                    )
                    # s = f[i+2]-f[i] = d[i]+d[i+1]; s4 = 0.25*s  (while
                    # ACT copies f_sh -- keeps DVE busy)
                    s = wk.tile([128, CMAX + 2], BF16, tag="s")
                    nc.vector.tensor_tensor(
                        s[:, 0:C + 2], f[:, 2:C + 4], f[:, 0:C + 2],
                        AluOpType.subtract,
                    )
                    nc.vector.tensor_scalar(
                        s[:, 0:C + 2], s[:, 0:C + 2], 0.25, None,
                        AluOpType.mult,
                    )
                    # ACT: even-aligned shifted copy f_sh[k] = f[k+1]
                    f_sh = wk.tile([128, SMAX - 1], BF16, tag="f_sh")
                    nc.scalar.activation(f_sh[:, 0:S - 1], f[:, 1:S], COPY)
                    # d[k] = f[k+1]-f[k];  d1[k] = f[k+2]-f[k+1]
                    d = wk.tile([128, CMAX + 2], BF16, tag="d")
                    nc.vector.tensor_tensor(
                        d[:, 0:C + 2], f_sh[:, 0:C + 2], f[:, 0:C + 2],
                        AluOpType.subtract,
                    )
                    d1 = wk.tile([128, CMAX + 2], BF16, tag="d1")
                    nc.vector.tensor_tensor(
                        d1[:, 0:C + 2], f[:, 2:C + 4], f_sh[:, 0:C + 2],
                        AluOpType.subtract,
                    )
                    # u = min(d,d1) -> lo in place; w = max(d,d1) -> hi
                    u = wk.tile([128, CMAX + 2], BF16, tag="u")
                    W2 = C + 2
                    nc.vector.tensor_tensor(
                        u[:, 0:W2], d[:, 0:W2], d1[:, 0:W2], AluOpType.min
                    )
                    w = wk.tile([128, CMAX + 2], BF16, tag="w")
                    nc.vector.tensor_tensor(
                        w[:, 0:W2], d[:, 0:W2], d1[:, 0:W2], AluOpType.max
                    )
                    nc.vector.tensor_tensor(
                        u[:, 0:W2], u[:, 0:W2], s[:, 0:W2], AluOpType.min
                    )
                    nc.vector.tensor_tensor(
                        w[:, 0:W2], w[:, 0:W2], s[:, 0:W2], AluOpType.max
                    )
                    # hi0 = min(hi,0) -> s;  hs = max(lo, hi0) -> d
                    nc.vector.tensor_scalar(
                        s[:, 0:W2], w[:, 0:W2], 0.0, None, AluOpType.min
                    )
                    hs = d
                    nc.vector.tensor_tensor(
                        hs[:, 0:W2], u[:, 0:W2], s[:, 0:W2], AluOpType.max
                    )
                    # p = f1 - hs;  q = f1 + hs   (f1[k] = f_sh[k], even)
                    p = wk.tile([128, CMAX + 2], BF16, tag="p")
                    nc.vector.tensor_tensor(
                        p[:, 0:W2], f_sh[:, 0:W2], hs[:, 0:W2],
                        AluOpType.subtract,
                    )
                    q = wk.tile([128, CMAX + 2], BF16, tag="q")
                    nc.vector.tensor_tensor(
                        q[:, 0:W2], f_sh[:, 0:W2], hs[:, 0:W2],
                        AluOpType.add,
                    )
                    # pm = p*mneg -> p;  qm = q*mpos -> q
                    nc.vector.tensor_tensor(
                        p[:, 0:W2], p[:, 0:W2], mneg[:, 0:W2], AluOpType.mult
                    )
                    nc.vector.tensor_tensor(
                        q[:, 0:W2], q[:, 0:W2], mpos[:, 0:W2], AluOpType.mult
                    )
                    # global boundary conditions
                    if c0 == 0:
                        nc.vector.memset(q[:, 0:1], 0.0)
                    if c0 + C == OUT_L:
                        nc.vector.memset(p[:, C + 1:C + 2], 0.0)
                    # fn = pm[1:] + qm[:-1]  (odd src -- degraded mode)
                    fn = u  # u (lo) is dead after hs
                    nc.vector.tensor_tensor(
                        fn[:, 0:C + 1], p[:, 1:C + 2], q[:, 0:C + 1],
                        AluOpType.add,
                    )
                    # out_b = fn[:-1] - fn[1:]  (odd src -- degraded mode)
                    out_b = w  # w (hi) is dead after hi0
                    nc.vector.tensor_tensor(
                        out_b[:, 0:C], fn[:, 0:C], fn[:, 1:C + 1],
                        AluOpType.subtract,
                    )
                    # ACT: final convert bf16 -> fp32
                    out_t = io.tile([128, CMAX], F32, tag="out")
                    nc.scalar.activation(out_t[:, 0:C], out_b[:, 0:C], COPY)
                    nc.sync.dma_start(
                        out[r0:r0 + 128, c0:c0 + C], out_t[:, 0:C]
                    )
                    c0 += C
    _split_multi_waits(nc)
    return nc


def build_module_v6(repeat=1, chunk=2046, wk_bufs=2, pool_ops=(),
                    abs_mode="mixed", masks="ts", f_mixed=False):
    """v6: abs/sign minmod (13 binary ops vs v4's 15), HWDGE-only (works
    inside For_i, so the graded module IS the timed module), Pool-engine
    offload for up to 3 shallow binaries, conversions on ACT.

    Math: d[k] = f[k+1]-f[k]; s = d[k]+d[k+1] = f[k+2]-f[k];
      hs = minmod3(d0, s/4, d1)
         = (sign(d0)+sign(d1)) * min(|d0|/2, |d1|/2, |s|/8)
      p = f1 - hs; q = f1 + hs; pm = (v1<0)*p; qm = (v1>0)*q
      fn = pm[1:] + qm; out = fn[:-1] - fn[1:]

    pool_ops: subset of {"s","pm","qm","q"} to run on the Pool engine.
    abs_on_act: compute |d|/2, |s|/8 on ACT (Abs w/ scale) vs DVE 2-op TS.
    masks: "ts" (DVE is_lt/is_gt @4x) or "act" (sign+relu pair on ACT).
    f_mixed: f = rho_b * v_t(f32) at 1x, dropping the v_b conversion
      (masks then read a pre-shifted ACT-converted v_sh instead).
    """
    import contextlib
    nc = bass.Bass()
    rho = nc.dram_tensor("rho", [ROWS, L], F32, kind="ExternalInput")
    vin = nc.dram_tensor("v", [ROWS, L], F32, kind="ExternalInput")
    out = nc.dram_tensor("out", [ROWS, OUT_L], F32, kind="ExternalOutput")

    SIGN = mybir.ActivationFunctionType.Sign
    RELU = mybir.ActivationFunctionType.Relu
    ABS = mybir.ActivationFunctionType.Abs

    n_chunks = max(1, round(OUT_L / chunk))
    base = (OUT_L // n_chunks) & ~1
    sizes = [base] * (n_chunks - 1) + [OUT_L - base * (n_chunks - 1)]
    assert all(sz % 2 == 0 for sz in sizes) and sum(sizes) == OUT_L, sizes
    CMAX = max(sizes)
    SMAX = CMAX + 4

    def vec_tt(dst, i0, i1, op, eng="dve"):
        if eng == "pool":
            nc.gpsimd.tensor_tensor(dst, i0, i1, op)
        else:
            nc.vector.tensor_tensor(dst, i0, i1, op)

    with TileContext(nc) as tc:
        with (
            tc.tile_pool(name="io", bufs=2) as io,
            tc.tile_pool(name="wk", bufs=wk_bufs) as wk,
            (tc.For_i(0, repeat, 1) if repeat > 1 else contextlib.nullcontext()),
        ):
            for rt in range(RT):
                r0 = rt * 128
                c0 = 0
                for C in sizes:
                    S = C + 4
                    W2 = C + 2
                    W3 = C + 3
                    peng = lambda name: "pool" if name in pool_ops else "dve"

                    # HWDGE fp32 loads
                    rho_t = io.tile([128, SMAX], F32, tag="rho")
                    nc.sync.dma_start(rho_t[:, 0:S], rho[r0:r0 + 128, c0:c0 + S])
                    v_t = io.tile([128, SMAX], F32, tag="v")
                    nc.sync.dma_start(v_t[:, 0:S], vin[r0:r0 + 128, c0:c0 + S])

                    # ACT conversions
                    rho_b = wk.tile([128, SMAX], BF16, tag="rho_b")
                    nc.scalar.activation(rho_b[:, 0:S], rho_t[:, 0:S], COPY)
                    if f_mixed:
                        v_sh = wk.tile([128, SMAX], BF16, tag="v_sh")
                        nc.scalar.activation(v_sh[:, 0:S - 1], v_t[:, 1:S],
                                             COPY)
                    else:
                        v_b = wk.tile([128, SMAX], BF16, tag="v_b")
                        nc.scalar.activation(v_b[:, 0:S], v_t[:, 0:S], COPY)

                    # f = rho * v
                    f = wk.tile([128, SMAX], BF16, tag="f")
                    if f_mixed:
                        nc.vector.tensor_tensor(f[:, 0:S], rho_b[:, 0:S],
                                                v_t[:, 0:S], AluOpType.mult)
                    else:
                        nc.vector.tensor_tensor(f[:, 0:S], rho_b[:, 0:S],
                                                v_b[:, 0:S], AluOpType.mult)
                    # ACT shifted copy f_sh[k] = f[k+1]
                    f_sh = wk.tile([128, SMAX - 1], BF16, tag="f_sh")
                    nc.scalar.activation(f_sh[:, 0:S - 1], f[:, 1:S], COPY)
                    # d[k] = f[k+1]-f[k]  [W3];  s[k] = f[k+2]-f[k]  [W2]
                    d = wk.tile([128, CMAX + 3], BF16, tag="d")
                    nc.vector.tensor_tensor(d[:, 0:W3], f_sh[:, 0:W3],
                                            f[:, 0:W3], AluOpType.subtract)
                    s = wk.tile([128, CMAX + 2], BF16, tag="s")
                    vec_tt(s[:, 0:W2], f[:, 2:C + 4], f[:, 0:W2],
                           AluOpType.subtract, peng("s"))
                    # abs magnitudes.  (Double-scalar TS mult+abs_max does
                    # not compile on this walrus -- verified -- so halving
                    # either rides ACT's input scale or a separate TS mult.)
                    #   act2 : a = |d|/2, as4 = |s|/8 on ACT; hs = t2*mn2
                    #   mixed: a = |d| TS; as4 = |s|/4 ACT; mh = mn2/2 TS
                    #   ts3  : a, as4 via 3 TS ops; mh = mn2/2 TS
                    a = wk.tile([128, CMAX + 3], BF16, tag="a")
                    as4 = wk.tile([128, CMAX + 2], BF16, tag="as4")
                    if abs_mode == "act2":
                        nc.scalar.activation(a[:, 0:W3], d[:, 0:W3], ABS,
                                             scale=0.5)
                        nc.scalar.activation(as4[:, 0:W2], s[:, 0:W2], ABS,
                                             scale=0.125)
                    elif abs_mode == "mixed":
                        nc.vector.tensor_scalar(a[:, 0:W3], d[:, 0:W3],
                                                0.0, None, AluOpType.abs_max)
                        nc.scalar.activation(as4[:, 0:W2], s[:, 0:W2], ABS,
                                             scale=0.25)
                    else:  # ts3
                        nc.vector.tensor_scalar(a[:, 0:W3], d[:, 0:W3],
                                                0.0, None, AluOpType.abs_max)
                        nc.vector.tensor_scalar(as4[:, 0:W2], s[:, 0:W2],
                                                0.0, None, AluOpType.abs_max)
                        nc.vector.tensor_scalar(as4[:, 0:W2], as4[:, 0:W2],
                                                0.25, None, AluOpType.mult)
                    # sg = sign(d) [W3]
                    sg = wk.tile([128, CMAX + 3], BF16, tag="sg")
                    nc.scalar.activation(sg[:, 0:W3], d[:, 0:W3], SIGN)
                    # mn = min(a, a[1:]) [W2];  mn2 = min(mn, as4)
                    mn = wk.tile([128, CMAX + 2], BF16, tag="mn")
                    nc.vector.tensor_tensor(mn[:, 0:W2], a[:, 0:W2],
                                            a[:, 1:W3], AluOpType.min)
                    mn2 = as4  # as4 dead after this
                    nc.vector.tensor_tensor(mn2[:, 0:W2], mn[:, 0:W2],
                                            as4[:, 0:W2], AluOpType.min)
                    # t2 = sg + sg[1:];  hs = t2 * (mn2 or mn2/2)
                    if abs_mode != "act2":
                        # halve once here (act2 pre-halves a/as4 instead)
                        nc.vector.tensor_scalar(mn2[:, 0:W2], mn2[:, 0:W2],
                                                0.5, None, AluOpType.mult)
                    t2 = d  # d dead after a/sg
                    nc.vector.tensor_tensor(t2[:, 0:W2], sg[:, 0:W2],
                                            sg[:, 1:W3], AluOpType.add)
                    hs = mn  # mn dead
                    nc.vector.tensor_tensor(hs[:, 0:W2], t2[:, 0:W2],
                                            mn2[:, 0:W2], AluOpType.mult)
                    # masks from v1 = v[c0+1 : c0+C+3]
                    mneg = wk.tile([128, CMAX + 2], BF16, tag="mneg")
                    mpos = wk.tile([128, CMAX + 2], BF16, tag="mpos")
                    if masks == "ts":
                        vsrc = v_sh[:, 0:W2] if f_mixed else v_b[:, 1:C + 3]
                        nc.vector.tensor_scalar(mneg[:, 0:W2], vsrc, 0.0,
                                                None, AluOpType.is_lt)
                        nc.vector.tensor_scalar(mpos[:, 0:W2], vsrc, 0.0,
                                                None, AluOpType.is_gt)
                    else:
                        sgv = wk.tile([128, CMAX + 2], BF16, tag="sgv")
                        vsrc = v_sh[:, 0:W2] if f_mixed else v_t[:, 1:C + 3]
                        nc.scalar.activation(sgv[:, 0:W2], vsrc, SIGN)
                        nc.scalar.activation(mneg[:, 0:W2], sgv[:, 0:W2],
                                             RELU, scale=-1.0)
                        nc.scalar.activation(mpos[:, 0:W2], sgv[:, 0:W2],
                                             RELU)
                    # p = f_sh - hs;  q = f_sh + hs
                    p = wk.tile([128, CMAX + 2], BF16, tag="p")
                    nc.vector.tensor_tensor(p[:, 0:W2], f_sh[:, 0:W2],
                                            hs[:, 0:W2], AluOpType.subtract)
                    q = wk.tile([128, CMAX + 2], BF16, tag="q")
                    vec_tt(q[:, 0:W2], f_sh[:, 0:W2], hs[:, 0:W2],
                           AluOpType.add, peng("q"))
                    # pm = p*mneg;  qm = q*mpos
                    vec_tt(p[:, 0:W2], p[:, 0:W2], mneg[:, 0:W2],
                           AluOpType.mult, peng("pm"))
                    vec_tt(q[:, 0:W2], q[:, 0:W2], mpos[:, 0:W2],
                           AluOpType.mult, peng("qm"))
                    if c0 == 0:
                        nc.vector.memset(q[:, 0:1], 0.0)
                    if c0 + C == OUT_L:
                        nc.vector.memset(p[:, C + 1:C + 2], 0.0)
                    # fn = pm[1:] + qm;  out_b = fn[:-1] - fn[1:]
                    fn = hs  # hs dead after p/q
                    nc.vector.tensor_tensor(fn[:, 0:C + 1], p[:, 1:C + 2],
                                            q[:, 0:C + 1], AluOpType.add)
                    out_b = p  # p dead
                    nc.vector.tensor_tensor(out_b[:, 0:C], fn[:, 0:C],
                                            fn[:, 1:C + 1],
                                            AluOpType.subtract)
                    # ACT out conversion + HWDGE store
                    out_t = io.tile([128, CMAX], F32, tag="out")
                    nc.scalar.activation(out_t[:, 0:C], out_b[:, 0:C], COPY)
                    nc.sync.dma_start(out[r0:r0 + 128, c0:c0 + C],
                                      out_t[:, 0:C])
                    c0 += C
    _split_multi_waits(nc)
    return nc


def build_module_v7(repeat=1, chunk=2730, wk_bufs=2, masks="ts",
                    relu_act=False, scale_act=False, rt_inner=False,
                    fuse_conv=False, f_f32=False, io_bufs=2,
                    masks_early=False, store_q="sp", taper=0, pad=0,
                    pqfuse=False, v_act_q=False, edge_split=False,
                    sgv_psum=False):
    """v7: minimal-sync DVE-centric variant (HWDGE-only, For_i-safe).

    Insights driving it (all HW-measured this session):
      * odd bf16 source offsets cost only ~+24% on TT (not 2x) -> shifted
        VIEWS beat materialized shifted copies; d1 = d[1:] is free.
      * clamp-form minmod saves a TT vs the min/max ladder:
          hs' = min(max(s'/4, min(m2',0)), relu(m1'))   [negated slopes]
        with m1' = min(d', d'[1:]), m2' = max(d', d'[1:]).
      * TS (tensor_scalar) mult/min/max/is_lt/is_gt run ~4x -> masks,
        0.25-scale, clamp bounds all stay on DVE, eliminating the ACT
        round-trips that dominated measured-vs-engine-busy gap.
      * cross-engine hops per chunk: DMA->ACT(conv x2)->DVE(whole
        chain)->ACT(out conv)->DMA.

    Negation bookkeeping (d' = -d0 = f[k]-f[k+1]):
      hs' = -hs;  p = f1 + hs';  q = f1 - hs'.
    """
    import contextlib
    nc = bass.Bass()
    rho = nc.dram_tensor("rho", [ROWS, L], F32, kind="ExternalInput")
    vin = nc.dram_tensor("v", [ROWS, L], F32, kind="ExternalInput")
    out = nc.dram_tensor("out", [ROWS, OUT_L], F32, kind="ExternalOutput")

    if taper:
        # small first/last chunks shrink the pipeline ramp (first DVE op
        # waits only for a small load+convert) and the drain tail.
        w0 = taper & ~1
        inner = OUT_L - 2 * w0
        n_in = max(1, round(inner / chunk))
        base = (inner // n_in) & ~1
        sizes = ([w0] + [base] * (n_in - 1)
                 + [inner - base * (n_in - 1)] + [w0])
    else:
        n_chunks = max(1, round(OUT_L / chunk))
        base = (OUT_L // n_chunks) & ~1
        sizes = [base] * (n_chunks - 1) + [OUT_L - base * (n_chunks - 1)]
    assert all(sz % 2 == 0 for sz in sizes) and sum(sizes) == OUT_L, sizes
    # pad widens every tile allocation without changing any computation
    # slice -- shifts the SBUF layout (bank-conflict lottery knob).
    CMAX = max(sizes) + pad
    SMAX = CMAX + 4

    with TileContext(nc) as tc:
        with (
            tc.tile_pool(name="io", bufs=io_bufs) as io,
            tc.tile_pool(name="wk", bufs=wk_bufs) as wk,
            (tc.psum_pool(name="ps", bufs=2) if sgv_psum
             else contextlib.nullcontext()) as ps,
            (tc.For_i(0, repeat, 1) if repeat > 1 else contextlib.nullcontext()),
        ):
            # rt_inner interleaves the four row-tiles at each chunk column,
            # giving the scheduler 4 independent chains to fill bubbles.
            units = [(rt, ci) for rt in range(RT) for ci in range(len(sizes))]
            if rt_inner:
                units = [(rt, ci) for ci in range(len(sizes))
                         for rt in range(RT)]
            offs = [sum(sizes[:i]) for i in range(len(sizes))]
            for rt, ci in units:
                    r0 = rt * 128
                    c0 = offs[ci]
                    C = sizes[ci]
                    S = C + 4
                    W2 = C + 2
                    W3 = C + 3
                    if fuse_conv:
                        # one f32 staging tile holds rho|v gap-free ->
                        # a single wide ACT conversion covers both.
                        rv_t = io.tile([128, 2 * SMAX], F32, tag="rv")
                        nc.sync.dma_start(rv_t[:, 0:S],
                                          rho[r0:r0 + 128, c0:c0 + S])
                        # v of the FIRST unit rides the ACT DMA queue so the
                        # two ramp-critical loads run concurrently (later
                        # units would queue behind ACT compute -- measured
                        # worse -- so only unit 0 is redirected).
                        first = (rt, ci) == units[0]
                        v_dma_eng = (nc.scalar if (v_act_q and first)
                                     else nc.sync)
                        v_dma_eng.dma_start(rv_t[:, S:2 * S],
                                            vin[r0:r0 + 128, c0:c0 + S])
                        rv_b = wk.tile([128, 2 * SMAX], BF16, tag="rv_b")
                        if edge_split and (rt, ci) == units[0]:
                            # ramp: split the first unit's conversion so the
                            # rho half runs while the v DMA is still in
                            # flight (loads are serial -- HBM-bound).
                            nc.scalar.activation(rv_b[:, 0:S],
                                                 rv_t[:, 0:S], COPY)
                            nc.scalar.activation(rv_b[:, S:2 * S],
                                                 rv_t[:, S:2 * S], COPY)
                        else:
                            nc.scalar.activation(rv_b[:, 0:2 * S],
                                                 rv_t[:, 0:2 * S], COPY)
                        rho_b = rv_b[:, 0:SMAX]
                        v_b = rv_b[:, S:S + SMAX]
                        v_t = rv_t[:, S:S + SMAX]
                    elif f_f32:
                        # no conversions: f multiplies the f32 staging
                        # tiles directly (1x DVE) -- frees ~27% of ACT.
                        # masks must be "act"/"hybrid" (no v_b): sgv reads
                        # v_t.
                        rho_t = io.tile([128, SMAX], F32, tag="rho")
                        nc.sync.dma_start(rho_t[:, 0:S],
                                          rho[r0:r0 + 128, c0:c0 + S])
                        v_t = io.tile([128, SMAX], F32, tag="v")
                        nc.sync.dma_start(v_t[:, 0:S],
                                          vin[r0:r0 + 128, c0:c0 + S])
                        rho_b = rho_t
                        v_b = v_t
                    else:
                        # HWDGE fp32 loads
                        rho_t = io.tile([128, SMAX], F32, tag="rho")
                        nc.sync.dma_start(rho_t[:, 0:S],
                                          rho[r0:r0 + 128, c0:c0 + S])
                        v_t = io.tile([128, SMAX], F32, tag="v")
                        nc.sync.dma_start(v_t[:, 0:S],
                                          vin[r0:r0 + 128, c0:c0 + S])
                        # ACT conversions (only engine hop before the chain)
                        rho_b = wk.tile([128, SMAX], BF16, tag="rho_b")
                        nc.scalar.activation(rho_b[:, 0:S], rho_t[:, 0:S],
                                             COPY)
                        v_b = wk.tile([128, SMAX], BF16, tag="v_b")
                        nc.scalar.activation(v_b[:, 0:S], v_t[:, 0:S], COPY)
                    # masks from v1 = v_b[1:C+3] / v_t[1:C+3].  With pqfuse,
                    # mneg|mpos live adjacently in one tile so the two mask
                    # multiplies later collapse into one double-width TT.
                    if pqfuse:
                        mm = wk.tile([128, 2 * (CMAX + 2)], BF16, tag="mm")
                        mneg = mm[:, 0:CMAX + 2]
                        mpos = mm[:, W2:W2 + CMAX + 2]
                    else:
                        mneg = wk.tile([128, CMAX + 2], BF16, tag="mneg")
                        mpos = wk.tile([128, CMAX + 2], BF16, tag="mpos")

                    def emit_masks():
                        if masks == "act":
                            SIGN = mybir.ActivationFunctionType.Sign
                            RELU = mybir.ActivationFunctionType.Relu
                            # sgv in PSUM keeps its 3 ACT accesses off the
                            # SBUF ports (probe for SBUF port contention
                            # with DVE); PSUM is also faster for ACT.
                            sgv_pool = ps if sgv_psum else wk
                            sgv = sgv_pool.tile([128, CMAX + 2], BF16,
                                                tag="sgv")
                            nc.scalar.activation(sgv[:, 0:W2],
                                                 v_t[:, 1:C + 3], SIGN)
                            nc.scalar.activation(mneg[:, 0:W2],
                                                 sgv[:, 0:W2], RELU,
                                                 scale=-1.0)
                            nc.scalar.activation(mpos[:, 0:W2],
                                                 sgv[:, 0:W2], RELU)
                        elif masks == "hybrid":
                            SIGN = mybir.ActivationFunctionType.Sign
                            RELU = mybir.ActivationFunctionType.Relu
                            sgv = wk.tile([128, CMAX + 2], BF16, tag="sgv")
                            nc.scalar.activation(sgv[:, 0:W2],
                                                 v_t[:, 1:C + 3], SIGN)
                            nc.scalar.activation(mpos[:, 0:W2],
                                                 sgv[:, 0:W2], RELU)
                            nc.vector.tensor_scalar(mneg[:, 0:W2],
                                                    v_b[:, 1:C + 3],
                                                    0.0, None,
                                                    AluOpType.is_lt)
                        else:
                            nc.vector.tensor_scalar(mneg[:, 0:W2],
                                                    v_b[:, 1:C + 3],
                                                    0.0, None,
                                                    AluOpType.is_lt)
                            nc.vector.tensor_scalar(mpos[:, 0:W2],
                                                    v_b[:, 1:C + 3],
                                                    0.0, None,
                                                    AluOpType.is_gt)

                    if masks_early:
                        emit_masks()
                    # f = rho*v
                    f = wk.tile([128, SMAX], BF16, tag="f")
                    nc.vector.tensor_tensor(f[:, 0:S], rho_b[:, 0:S],
                                            v_b[:, 0:S], AluOpType.mult)
                    # d'[k] = f[k] - f[k+1]  [W3]; d1' is the view d'[1:]
                    d = wk.tile([128, CMAX + 3], BF16, tag="d")
                    nc.vector.tensor_tensor(d[:, 0:W3], f[:, 0:W3],
                                            f[:, 1:S], AluOpType.subtract)
                    # s' = f - f[2:]  [W2]; b' = s'/4 in place
                    s = wk.tile([128, CMAX + 2], BF16, tag="s")
                    nc.vector.tensor_tensor(s[:, 0:W2], f[:, 0:W2],
                                            f[:, 2:C + 4], AluOpType.subtract)
                    if scale_act:
                        nc.scalar.activation(s[:, 0:W2], s[:, 0:W2], COPY,
                                             scale=0.25)
                    else:
                        nc.vector.tensor_scalar(s[:, 0:W2], s[:, 0:W2], 0.25,
                                                None, AluOpType.mult)
                    # m1' = min(d', d'[1:]);  m2' = max(d', d'[1:])
                    m1 = wk.tile([128, CMAX + 2], BF16, tag="m1")
                    nc.vector.tensor_tensor(m1[:, 0:W2], d[:, 0:W2],
                                            d[:, 1:W3], AluOpType.min)
                    m2 = wk.tile([128, CMAX + 2], BF16, tag="m2")
                    nc.vector.tensor_tensor(m2[:, 0:W2], d[:, 0:W2],
                                            d[:, 1:W3], AluOpType.max)
                    # A = min(m2', 0) in place;  R = relu(m1') in place
                    nc.vector.tensor_scalar(m2[:, 0:W2], m2[:, 0:W2], 0.0,
                                            None, AluOpType.min)
                    if relu_act:
                        nc.scalar.activation(
                            m1[:, 0:W2], m1[:, 0:W2],
                            mybir.ActivationFunctionType.Relu)
                    else:
                        nc.vector.tensor_scalar(m1[:, 0:W2], m1[:, 0:W2],
                                                0.0, None, AluOpType.max)
                    # t1 = max(b', A) -> s;  hs' = min(t1, R) -> m1
                    nc.vector.tensor_tensor(s[:, 0:W2], s[:, 0:W2],
                                            m2[:, 0:W2], AluOpType.max)
                    hs = m1
                    nc.vector.tensor_tensor(hs[:, 0:W2], s[:, 0:W2],
                                            m1[:, 0:W2], AluOpType.min)
                    # p = f[1:] + hs';  q = f[1:] - hs'
                    if pqfuse:
                        pq = wk.tile([128, 2 * (CMAX + 2)], BF16, tag="pq")
                        p = pq[:, 0:CMAX + 2]
                        q = pq[:, W2:W2 + CMAX + 2]
                    else:
                        p = wk.tile([128, CMAX + 2], BF16, tag="p")
                        q = wk.tile([128, CMAX + 2], BF16, tag="q")
                    nc.vector.tensor_tensor(p[:, 0:W2], f[:, 1:C + 3],
                                            hs[:, 0:W2], AluOpType.add)
                    nc.vector.tensor_tensor(q[:, 0:W2], f[:, 1:C + 3],
                                            hs[:, 0:W2], AluOpType.subtract)
                    if not masks_early:
                        emit_masks()
                    # pm = p*mneg -> p;  qm = q*mpos -> q.  With pqfuse the
                    # two multiplies are one double-width TT over pq|mm.
                    if pqfuse:
                        nc.vector.tensor_tensor(pq[:, 0:2 * W2],
                                                pq[:, 0:2 * W2],
                                                mm[:, 0:2 * W2],
                                                AluOpType.mult)
                    else:
                        nc.vector.tensor_tensor(p[:, 0:W2], p[:, 0:W2],
                                                mneg[:, 0:W2],
                                                AluOpType.mult)
                        nc.vector.tensor_tensor(q[:, 0:W2], q[:, 0:W2],
                                                mpos[:, 0:W2],
                                                AluOpType.mult)
                    if c0 == 0:
                        nc.vector.memset(q[:, 0:1], 0.0)
                    if c0 + C == OUT_L:
                        nc.vector.memset(p[:, C + 1:C + 2], 0.0)
                    # fn = pm[1:] + qm -> d (dead);  out_b = fn - fn[1:] -> s
                    fn = d
                    nc.vector.tensor_tensor(fn[:, 0:C + 1], p[:, 1:C + 2],
                                            q[:, 0:C + 1], AluOpType.add)
                    out_b = s
                    nc.vector.tensor_tensor(out_b[:, 0:C], fn[:, 0:C],
                                            fn[:, 1:C + 1],
                                            AluOpType.subtract)
                    # ACT out conversion + HWDGE store
                    out_t = io.tile([128, CMAX], F32, tag="out")
                    store_eng = nc.vector if store_q == "dve" else nc.sync
                    if edge_split and (rt, ci) == units[-1]:
                        # tail: halve the last unit's convert+store so the
                        # first store overlaps the second conversion.
                        H = (C // 2) & ~1
                        nc.scalar.activation(out_t[:, 0:H], out_b[:, 0:H],
                                             COPY)
                        store_eng.dma_start(out[r0:r0 + 128, c0:c0 + H],
                                            out_t[:, 0:H])
                        nc.scalar.activation(out_t[:, H:C], out_b[:, H:C],
                                             COPY)
                        store_eng.dma_start(
                            out[r0:r0 + 128, c0 + H:c0 + C], out_t[:, H:C])
                    else:
                        nc.scalar.activation(out_t[:, 0:C], out_b[:, 0:C],
                                             COPY)
                        store_eng.dma_start(out[r0:r0 + 128, c0:c0 + C],
                                            out_t[:, 0:C])
                    c0 += C
    _split_multi_waits(nc)
    return nc


# Shipped configuration: v7 (see build_module_v7 docstring).  The same
# module is used for the graded kernel() call and (with repeat>1) for the
# For_i device-time measurement in test.py -- HWDGE-only, so it compiles
# inside the hardware loop.
V7_CFG = dict(chunk=2046, masks="act", relu_act=True, scale_act=True,
              fuse_conv=True, rt_inner=True, pqfuse=True, edge_split=True)

_NC_CACHE = None


def _get_nc():
    global _NC_CACHE
    if _NC_CACHE is None:
        _NC_CACHE = build_module_v7(**V7_CFG)
    return _NC_CACHE


def kernel(rho, v, axis=2, retain_padding=0, **_kw):
    rho = np.ascontiguousarray(np.asarray(rho, dtype=np.float32))
    v = np.ascontiguousarray(np.asarray(v, dtype=np.float32))
    assert rho.shape == (B, M, L) and v.shape == (B, M, L)

    nc = _get_nc()
    in_maps = [
        {
            "rho": rho[c * BP:(c + 1) * BP].reshape(ROWS, L),
            "v": v[c * BP:(c + 1) * BP].reshape(ROWS, L),
        }
        for c in range(NCORES)
    ]
    last_err = None
    for _attempt in range(3):
        try:
            res = run_bass_kernel_spmd(
                nc, in_maps, core_ids=list(range(NCORES))
            )
            break
        except Exception as e:  # rare transient NRT device errors
            last_err = e
            import time as _time
            _time.sleep(5)
    else:
        raise last_err
    outs = [r["out"].reshape(BP, M, OUT_L) for r in res.results]
    return np.concatenate(outs, axis=0)


def build_module_v5(repeat=1, chunk=1636, wk_bufs=2):
    """v5: whole-input-resident variant.

    Phase 1 (outside the repeat loop): SWDGE cast-loads of ALL of rho/v
    into persistent SBUF bf16 buffers (16.8 MB of 28 MB SBUF).
    Phase 2 (loopable): compute + ACT out-conversion + HWDGE fp32 store
    -- contains no SWDGE, so it compiles inside For_i for benchmarking.
    """
    import contextlib
    nc = bass.Bass()
    rho = nc.dram_tensor("rho", [ROWS, L], F32, kind="ExternalInput")
    vin = nc.dram_tensor("v", [ROWS, L], F32, kind="ExternalInput")
    out = nc.dram_tensor("out", [ROWS, OUT_L], F32, kind="ExternalOutput")

    SIGN = mybir.ActivationFunctionType.Sign
    RELU = mybir.ActivationFunctionType.Relu

    n_chunks = max(1, round(OUT_L / chunk))
    base = (OUT_L // n_chunks) & ~1
    sizes = [base] * (n_chunks - 1) + [OUT_L - base * (n_chunks - 1)]
    assert all(sz % 2 == 0 for sz in sizes) and sum(sizes) == OUT_L, sizes
    CMAX = max(sizes)

    with TileContext(nc) as tc:
        with (
            tc.tile_pool(name="inp", bufs=1) as inp,
            tc.tile_pool(name="io", bufs=2) as io,
            tc.tile_pool(name="wk", bufs=wk_bufs) as wk,
        ):
            rho_sb = inp.tile([128, RT, L], BF16, tag="rho_sb")
            v_sb = inp.tile([128, RT, L], BF16, tag="v_sb")
            for rt in range(RT):
                r0 = rt * 128
                nc.gpsimd.dma_start(rho_sb[:, rt, :], rho[r0:r0 + 128, :])
                nc.gpsimd.dma_start(v_sb[:, rt, :], vin[r0:r0 + 128, :])
            with (tc.For_i(0, repeat, 1) if repeat > 1
                  else contextlib.nullcontext()):
                for rt in range(RT):
                    r0 = rt * 128
                    c0 = 0
                    for C in sizes:
                        S = C + 4
                        W2 = C + 2
                        rv = rho_sb[:, rt, c0:c0 + S]
                        vv = v_sb[:, rt, c0:c0 + S]
                        # ACT masks: sgn = sign(v1); mneg = relu(-sgn);
                        # mpos = relu(sgn) in-place.
                        sgn = wk.tile([128, CMAX + 2], BF16, tag="sgn")
                        nc.scalar.activation(
                            sgn[:, 0:W2], v_sb[:, rt, c0 + 1:c0 + C + 3],
                            SIGN,
                        )
                        mneg = wk.tile([128, CMAX + 2], BF16, tag="mneg")
                        nc.scalar.activation(
                            mneg[:, 0:W2], sgn[:, 0:W2], RELU, scale=-1.0
                        )
                        mpos = sgn
                        nc.scalar.activation(
                            mpos[:, 0:W2], sgn[:, 0:W2], RELU
                        )
                        # DVE: f = rho*v
                        f = wk.tile([128, CMAX + 4], BF16, tag="f")
                        nc.vector.tensor_tensor(
                            f[:, 0:S], rv, vv, AluOpType.mult
                        )
                        # s' = f[i]-f[i+2]
                        s = wk.tile([128, CMAX + 2], BF16, tag="s")
                        nc.vector.tensor_tensor(
                            s[:, 0:W2], f[:, 0:W2], f[:, 2:C + 4],
                            AluOpType.subtract,
                        )
                        # ACT: f_sh[k] = f[k+1];  s4' = 0.25*s'
                        f_sh = wk.tile([128, CMAX + 3], BF16, tag="f_sh")
                        nc.scalar.activation(
                            f_sh[:, 0:S - 1], f[:, 1:S], COPY
                        )
                        nc.scalar.activation(
                            s[:, 0:W2], s[:, 0:W2], COPY, scale=0.25
                        )
                        # d' = f - f_sh;  d1' = f_sh - f[2:]
                        d = wk.tile([128, CMAX + 2], BF16, tag="d")
                        nc.vector.tensor_tensor(
                            d[:, 0:W2], f[:, 0:W2], f_sh[:, 0:W2],
                            AluOpType.subtract,
                        )
                        d1 = wk.tile([128, CMAX + 2], BF16, tag="d1")
                        nc.vector.tensor_tensor(
                            d1[:, 0:W2], f_sh[:, 0:W2], f[:, 2:C + 4],
                            AluOpType.subtract,
                        )
                        # u' = max(d',d1') -> nlo;  w' = min(d',d1') -> nhi
                        u = wk.tile([128, CMAX + 2], BF16, tag="u")
                        nc.vector.tensor_tensor(
                            u[:, 0:W2], d[:, 0:W2], d1[:, 0:W2],
                            AluOpType.max,
                        )
                        w = wk.tile([128, CMAX + 2], BF16, tag="w")
                        nc.vector.tensor_tensor(
                            w[:, 0:W2], d[:, 0:W2], d1[:, 0:W2],
                            AluOpType.min,
                        )
                        nc.vector.tensor_tensor(
                            u[:, 0:W2], u[:, 0:W2], s[:, 0:W2],
                            AluOpType.max,
                        )
                        nc.vector.tensor_tensor(
                            w[:, 0:W2], w[:, 0:W2], s[:, 0:W2],
                            AluOpType.min,
                        )
                        # ACT: nhi0 = relu(nhi) -> s
                        nc.scalar.activation(s[:, 0:W2], w[:, 0:W2], RELU)
                        # hs' = min(nlo, nhi0) -> d
                        hs = d
                        nc.vector.tensor_tensor(
                            hs[:, 0:W2], u[:, 0:W2], s[:, 0:W2],
                            AluOpType.min,
                        )
                        # p = f_sh + hs';  q = f_sh - hs'
                        p = wk.tile([128, CMAX + 2], BF16, tag="p")
                        nc.vector.tensor_tensor(
                            p[:, 0:W2], f_sh[:, 0:W2], hs[:, 0:W2],
                            AluOpType.add,
                        )
                        q = wk.tile([128, CMAX + 2], BF16, tag="q")
                        nc.vector.tensor_tensor(
                            q[:, 0:W2], f_sh[:, 0:W2], hs[:, 0:W2],
                            AluOpType.subtract,
                        )
                        # pm, qm
                        nc.vector.tensor_tensor(
                            p[:, 0:W2], p[:, 0:W2], mneg[:, 0:W2],
                            AluOpType.mult,
                        )
                        nc.vector.tensor_tensor(
                            q[:, 0:W2], q[:, 0:W2], mpos[:, 0:W2],
                            AluOpType.mult,
                        )
                        if c0 == 0:
                            nc.vector.memset(q[:, 0:1], 0.0)
                        if c0 + C == OUT_L:
                            nc.vector.memset(p[:, C + 1:C + 2], 0.0)
                        # fn = pm[1:] + qm[:-1];  out_b = fn[:-1]-fn[1:]
                        fn = u
                        nc.vector.tensor_tensor(
                            fn[:, 0:C + 1], p[:, 1:C + 2], q[:, 0:C + 1],
                            AluOpType.add,
                        )
                        out_b = w
                        nc.vector.tensor_tensor(
                            out_b[:, 0:C], fn[:, 0:C], fn[:, 1:C + 1],
                            AluOpType.subtract,
                        )
                        # ACT: convert bf16 -> fp32; HWDGE store (loop-safe)
                        out_t = io.tile([128, CMAX], F32, tag="out")
                        nc.scalar.activation(
                            out_t[:, 0:C], out_b[:, 0:C], COPY
                        )
                        nc.sync.dma_start(
                            out[r0:r0 + 128, c0:c0 + C], out_t[:, 0:C]
                        )
                        c0 += C
    _split_multi_waits(nc)
    return nc



# revision 45
# speedup vs baseline: 1.0056x; 1.0056x over previous
"""Trainium2 Bass kernel for the 1D advection stencil (slope-limited flux).

Math (axis=-1, L = N + 4 ghost cells, th = 2.0):
    flux = rho * v
    d[i]  = flux[i+1] - flux[i]
    hs[i] = minmod3(d[i], (d[i]+d[i+1])/4, d[i+1])        # == 0.5*minmod3(c0,c1,c2)
    p[i]  = flux[i+1] - hs[i];  q[i] = flux[i+1] + hs[i]
    pm[i] = (v[i+1] < 0) * p[i];  qm[i] = (v[i+1] > 0) * q[i]
    pm[L-3] = 0; qm[0] = 0
    fn[j]  = pm[j+1] + qm[j]
    out[i] = fn[i] - fn[i+1]
minmod3(a,b,c) = max(min3, min(max3, 0)).

SHIPPED: build_module_v7 (see its docstring) -- clamp-form minmod,
13 tensor_tensor ops/chunk, shifted views instead of materialized shifted
copies, HWDGE-only so the graded module also runs inside tc.For_i for the
device-time measurement.  Older builders (v3/v4/v5/v6) are kept for
reference/ablation only.

Implementation notes (measured/derived on this container):
  * DVE rates: tensor_tensor fp32 = 1x, bf16 = 2x (2x_1p, needs 4B-aligned
    starts: odd bf16 element offsets degrade); tensor_scalar bf16 = 4x;
    scalar_tensor_tensor = 1x ALWAYS (no fast uops -> avoided entirely).
  * rel-err budget is 2e-2; full bf16 pipeline measures ~3.8e-3.
  * ACT (scalar engine, 1x @1.2GHz) does dtype conversions, the
    even-aligned shifted copy of f, and the v-sign masks off the DVE
    critical path. ACT bias/scale are per-partition scalars only.
  * Custom fused DVE ops do NOT compile here (walrus codegen rejects
    InstCustomDveAnt: "ISA wrong length") -- verified.
  * Pool/GpSimd shares an SBUF port with DVE (exclusive lock) -- unused.

Sharding: pure data-parallel over the leading batch axis B=16 -> 2 slabs
per core on 8 cores.  No halo exchange needed.
"""

import numpy as np

import concourse.bass as bass
import concourse.mybir as mybir
from concourse.mybir import AluOpType
from concourse.tile import TileContext
from concourse.bass_utils import run_bass_kernel_spmd

# Problem shape (hardcoded; kernel.py must be self-contained).
B, M, L = 16, 256, 8192
NCORES = 8
BP = B // NCORES            # 2 batch slabs per core
ROWS = BP * M               # 512 rows per core
RT = ROWS // 128            # 4 partition tiles of 128 rows
OUT_L = L - 4               # 8188
F32 = mybir.dt.float32
BF16 = mybir.dt.bfloat16
COPY = mybir.ActivationFunctionType.Copy


def _split_multi_waits(nc):
    """Walrus in this environment rejects instructions carrying more than
    one sync wait ("Too many sync wait commands").  Tile freely attaches
    several.  Split: for an instruction with k>1 waits, emit k-1 engine
    NoOps (one wait each) immediately before it, leaving one wait on the
    instruction itself."""
    import copy
    import concourse.mybir as mybir

    counter = [0]

    def mk_nop(engine, wait):
        counter[0] += 1
        return mybir.InstNoOp(
            name=f"waitsplit-{counter[0]}",
            engine=engine,
            ins=[],
            outs=[],
            sync_info=mybir.SyncInfo(on_wait=[wait], on_update=[]),
        )

    m = nc.m
    new_module = copy.replace(m, functions=[])
    for function in m.functions:
        new_function = copy.replace(function, blocks=[])
        new_function.set_allocations_from_list(function.allocations)
        for block in function.blocks:
            new_insts = []
            for inst in block.instructions:
                si = inst.sync_info
                waits = list(si.on_wait) if (si and si.on_wait) else []
                if len(waits) > 1:
                    for w in waits[:-1]:
                        new_insts.append(mk_nop(inst.engine, w))
                    inst.sync_info = mybir.SyncInfo(
                        on_wait=[waits[-1]], on_update=list(si.on_update)
                    )
                new_insts.append(inst)
            new_function.blocks.append(
                copy.replace(block, instructions=new_insts)
            )
        new_module.functions.append(new_function)
    nc.m = new_module


def _emit_v4_chunk(nc, wk, rho, vin, out, r0, c0, C, CMAX, SMAX,
                   act_pm_sh=False, act_fn_sh=False):
    """v4 chunk body: casting SWDGE DMAs for all dtype conversions (no
    fp32 staging tiles, no ACT conversion passes) + the negated minmod
    chain so the unary steps (s4 scale, relu(-hi)) run on ACT.

    Negation bookkeeping: d' = -d, d1' = -d1, s' = -s, u' = -u, w' = -w,
    nlo = -lo, nhi = -hi, nhi0 = relu(nhi) = -min(hi, 0), hs' = -hs.
    Then p = f1 + hs' (add) and q = f1 - hs' (subtract)."""
    SIGN = mybir.ActivationFunctionType.Sign
    RELU = mybir.ActivationFunctionType.Relu
    S = C + 4
    W2 = C + 2

    # Casting loads (SWDGE): DRAM fp32 -> SBUF bf16.
    rho_b = wk.tile([128, SMAX], BF16, tag="rho_b")
    nc.gpsimd.dma_start(rho_b[:, 0:S], rho[r0:r0 + 128, c0:c0 + S])
    v_b = wk.tile([128, SMAX], BF16, tag="v_b")
    nc.gpsimd.dma_start(v_b[:, 0:S], vin[r0:r0 + 128, c0:c0 + S])

    # ACT masks from v_b: sgn = sign(v1); mneg = relu(-sgn) = (v1<0);
    # mpos = relu(sgn) = (v1>0) in-place over sgn.
    sgn = wk.tile([128, CMAX + 2], BF16, tag="sgn")
    nc.scalar.activation(sgn[:, 0:W2], v_b[:, 1:C + 3], SIGN)
    mneg = wk.tile([128, CMAX + 2], BF16, tag="mneg")
    nc.scalar.activation(mneg[:, 0:W2], sgn[:, 0:W2], RELU, scale=-1.0)
    mpos = sgn
    nc.scalar.activation(mpos[:, 0:W2], sgn[:, 0:W2], RELU)

    # DVE: f = rho*v
    f = wk.tile([128, SMAX], BF16, tag="f")
    nc.vector.tensor_tensor(f[:, 0:S], rho_b[:, 0:S], v_b[:, 0:S],
                            AluOpType.mult)
    # s' = f[i] - f[i+2] = -(d0+d1) while ACT copies f_sh
    s = wk.tile([128, CMAX + 2], BF16, tag="s")
    nc.vector.tensor_tensor(s[:, 0:W2], f[:, 0:W2], f[:, 2:C + 4],
                            AluOpType.subtract)
    # ACT: even-aligned shifted copy f_sh[k] = f[k+1]
    f_sh = wk.tile([128, SMAX - 1], BF16, tag="f_sh")
    nc.scalar.activation(f_sh[:, 0:S - 1], f[:, 1:S], COPY)
    # ACT: s4' = 0.25 * s'
    nc.scalar.activation(s[:, 0:W2], s[:, 0:W2], COPY, scale=0.25)
    # d' = f - f_sh;  d1' = f_sh - f[2:]
    d = wk.tile([128, CMAX + 2], BF16, tag="d")
    nc.vector.tensor_tensor(d[:, 0:W2], f[:, 0:W2], f_sh[:, 0:W2],
                            AluOpType.subtract)
    d1 = wk.tile([128, CMAX + 2], BF16, tag="d1")
    nc.vector.tensor_tensor(d1[:, 0:W2], f_sh[:, 0:W2], f[:, 2:C + 4],
                            AluOpType.subtract)
    # u' = max(d', d1') = -u;  w' = min(d', d1') = -w
    u = wk.tile([128, CMAX + 2], BF16, tag="u")
    nc.vector.tensor_tensor(u[:, 0:W2], d[:, 0:W2], d1[:, 0:W2],
                            AluOpType.max)
    w = wk.tile([128, CMAX + 2], BF16, tag="w")
    nc.vector.tensor_tensor(w[:, 0:W2], d[:, 0:W2], d1[:, 0:W2],
                            AluOpType.min)
    # nlo = max(u', s4') = -lo;  nhi = min(w', s4') = -hi
    nc.vector.tensor_tensor(u[:, 0:W2], u[:, 0:W2], s[:, 0:W2],
                            AluOpType.max)
    nc.vector.tensor_tensor(w[:, 0:W2], w[:, 0:W2], s[:, 0:W2],
                            AluOpType.min)
    # ACT: nhi0 = relu(nhi) = -min(hi,0)   (into s; s4' is dead)
    nc.scalar.activation(s[:, 0:W2], w[:, 0:W2], RELU)
    # hs' = min(nlo, nhi0) = -hs   (into d; d' is dead)
    hs = d
    nc.vector.tensor_tensor(hs[:, 0:W2], u[:, 0:W2], s[:, 0:W2],
                            AluOpType.min)
    # p = f1 - hs = f_sh + hs';  q = f1 + hs = f_sh - hs'
    p = wk.tile([128, CMAX + 2], BF16, tag="p")
    nc.vector.tensor_tensor(p[:, 0:W2], f_sh[:, 0:W2], hs[:, 0:W2],
                            AluOpType.add)
    q = wk.tile([128, CMAX + 2], BF16, tag="q")
    nc.vector.tensor_tensor(q[:, 0:W2], f_sh[:, 0:W2], hs[:, 0:W2],
                            AluOpType.subtract)
    # pm = p*mneg;  qm = q*mpos
    nc.vector.tensor_tensor(p[:, 0:W2], p[:, 0:W2], mneg[:, 0:W2],
                            AluOpType.mult)
    nc.vector.tensor_tensor(q[:, 0:W2], q[:, 0:W2], mpos[:, 0:W2],
                            AluOpType.mult)
    if c0 == 0:
        nc.vector.memset(q[:, 0:1], 0.0)
    if c0 + C == OUT_L:
        nc.vector.memset(p[:, C + 1:C + 2], 0.0)
    # fn = pm[1:] + qm[:-1]; out_b = fn[:-1] - fn[1:]
    if act_pm_sh:
        # even-aligned shifted copy of pm on ACT
        pm_sh = wk.tile([128, CMAX + 1], BF16, tag="pm_sh")
        nc.scalar.activation(pm_sh[:, 0:C + 1], p[:, 1:C + 2], COPY)
        fn = u
        nc.vector.tensor_tensor(fn[:, 0:C + 1], pm_sh[:, 0:C + 1],
                                q[:, 0:C + 1], AluOpType.add)
    else:
        fn = u
        nc.vector.tensor_tensor(fn[:, 0:C + 1], p[:, 1:C + 2],
                                q[:, 0:C + 1], AluOpType.add)
    out_b = w
    if act_fn_sh:
        fn_sh = wk.tile([128, CMAX], BF16, tag="fn_sh")
        nc.scalar.activation(fn_sh[:, 0:C], fn[:, 1:C + 1], COPY)
        nc.vector.tensor_tensor(out_b[:, 0:C], fn[:, 0:C], fn_sh[:, 0:C],
                                AluOpType.subtract)
    else:
        nc.vector.tensor_tensor(out_b[:, 0:C], fn[:, 0:C], fn[:, 1:C + 1],
                                AluOpType.subtract)
    # Casting store (SWDGE): SBUF bf16 -> DRAM fp32.
    nc.gpsimd.dma_start(out[r0:r0 + 128, c0:c0 + C], out_b[:, 0:C])


def build_module(repeat=1, variant="v4", chunk=2730, wk_bufs=2,
                 act_pm_sh=False, act_fn_sh=False):
    """repeat>1 wraps the whole body in a device-side For_i loop --
    benchmark-only, so device time dominates the axon tunnel overhead.

    variant: "v3" (default), "dma" (transfers only -- roofline probe)."""
    import contextlib
    nc = bass.Bass()
    rho = nc.dram_tensor("rho", [ROWS, L], F32, kind="ExternalInput")
    vin = nc.dram_tensor("v", [ROWS, L], F32, kind="ExternalInput")
    out = nc.dram_tensor("out", [ROWS, OUT_L], F32, kind="ExternalOutput")

    SIGN = mybir.ActivationFunctionType.Sign
    RELU = mybir.ActivationFunctionType.Relu

    # All-even chunk sizes covering OUT_L (odd starts degrade bf16 packing,
    # odd lengths break the TS 4x even-dim requirement).
    n_chunks = max(1, round(OUT_L / chunk))
    base = (OUT_L // n_chunks) & ~1
    sizes = [base] * (n_chunks - 1) + [OUT_L - base * (n_chunks - 1)]
    assert all(sz % 2 == 0 for sz in sizes) and sum(sizes) == OUT_L, sizes
    CMAX = max(sizes)
    SMAX = CMAX + 4

    with TileContext(nc) as tc:
        with (
            tc.tile_pool(name="io", bufs=2) as io,
            tc.tile_pool(name="wk", bufs=wk_bufs) as wk,
            (tc.For_i(0, repeat, 1) if repeat > 1 else contextlib.nullcontext()),
        ):
            for rt in range(RT):
                r0 = rt * 128
                c0 = 0
                for C in sizes:
                    S = C + 4
                    if variant == "v4":
                        _emit_v4_chunk(nc, wk, rho, vin, out, r0, c0, C,
                                       CMAX, SMAX,
                                       act_pm_sh=act_pm_sh,
                                       act_fn_sh=act_fn_sh)
                        c0 += C
                        continue
                    rho_t = io.tile([128, SMAX], F32, tag="rho")
                    nc.sync.dma_start(
                        rho_t[:, 0:S], rho[r0:r0 + 128, c0:c0 + S]
                    )
                    v_t = io.tile([128, SMAX], F32, tag="v")
                    nc.sync.dma_start(
                        v_t[:, 0:S], vin[r0:r0 + 128, c0:c0 + S]
                    )
                    if variant == "dma":
                        out_t = io.tile([128, CMAX], F32, tag="out")
                        nc.vector.tensor_tensor(
                            out_t[:, 0:1], rho_t[:, 0:1], v_t[:, 0:1],
                            AluOpType.mult,
                        )
                        nc.sync.dma_start(
                            out[r0:r0 + 128, c0:c0 + C], out_t[:, 0:C]
                        )
                        c0 += C
                        continue
                    if variant == "v4":
                        _emit_v4_chunk(nc, wk, rho, vin, out, r0, c0, C,
                                       CMAX, SMAX)
                        c0 += C
                        continue

                    # --- ACT: conversions + masks (off-DVE) -------------
                    rho_b = wk.tile([128, SMAX], BF16, tag="rho_b")
                    nc.scalar.activation(rho_b[:, 0:S], rho_t[:, 0:S], COPY)
                    v_b = wk.tile([128, SMAX], BF16, tag="v_b")
                    nc.scalar.activation(v_b[:, 0:S], v_t[:, 0:S], COPY)
                    assert variant == "v3", variant
                    # sgn = sign(v1); masks mpos = relu(sgn) = (v1>0),
                    # mneg = relu(-sgn) = (v1<0).  All even-aligned.
                    sgn = wk.tile([128, CMAX + 2], BF16, tag="sgn")
                    nc.scalar.activation(sgn[:, 0:C + 2], v_t[:, 1:C + 3], SIGN)
                    # mneg = relu(-sgn) first, then mpos = relu(sgn)
                    # in-place over sgn (saves a tile tag).
                    mneg = wk.tile([128, CMAX + 2], BF16, tag="mneg")
                    nc.scalar.activation(
                        mneg[:, 0:C + 2], sgn[:, 0:C + 2], RELU, scale=-1.0
                    )
                    mpos = sgn
                    nc.scalar.activation(
                        mpos[:, 0:C + 2], sgn[:, 0:C + 2], RELU
                    )

                    # --- DVE bf16 pipeline ------------------------------
                    # f = rho*v  [S]
                    f = wk.tile([128, SMAX], BF16, tag="f")
                    nc.vector.tensor_tensor(
                        f[:, 0:S], rho_b[:, 0:S], v_b[:, 0:S], AluOpType.mult
                    )
                    # s = f[i+2]-f[i] = d[i]+d[i+1]; s4 = 0.25*s  (while
                    # ACT copies f_sh -- keeps DVE busy)
                    s = wk.tile([128, CMAX + 2], BF16, tag="s")
                    nc.vector.tensor_tensor(
                        s[:, 0:C + 2], f[:, 2:C + 4], f[:, 0:C + 2],
                        AluOpType.subtract,
                    )
                    nc.vector.tensor_scalar(
                        s[:, 0:C + 2], s[:, 0:C + 2], 0.25, None,
                        AluOpType.mult,
                    )
                    # ACT: even-aligned shifted copy f_sh[k] = f[k+1]
                    f_sh = wk.tile([128, SMAX - 1], BF16, tag="f_sh")
                    nc.scalar.activation(f_sh[:, 0:S - 1], f[:, 1:S], COPY)
                    # d[k] = f[k+1]-f[k];  d1[k] = f[k+2]-f[k+1]
                    d = wk.tile([128, CMAX + 2], BF16, tag="d")
                    nc.vector.tensor_tensor(
                        d[:, 0:C + 2], f_sh[:, 0:C + 2], f[:, 0:C + 2],
                        AluOpType.subtract,
                    )
                    d1 = wk.tile([128, CMAX + 2], BF16, tag="d1")
                    nc.vector.tensor_tensor(
                        d1[:, 0:C + 2], f[:, 2:C + 4], f_sh[:, 0:C + 2],
                        AluOpType.subtract,
                    )
                    # u = min(d,d1) -> lo in place; w = max(d,d1) -> hi
                    u = wk.tile([128, CMAX + 2], BF16, tag="u")
                    W2 = C + 2
                    nc.vector.tensor_tensor(
                        u[:, 0:W2], d[:, 0:W2], d1[:, 0:W2], AluOpType.min
                    )
                    w = wk.tile([128, CMAX + 2], BF16, tag="w")
                    nc.vector.tensor_tensor(
                        w[:, 0:W2], d[:, 0:W2], d1[:, 0:W2], AluOpType.max
                    )
                    nc.vector.tensor_tensor(
                        u[:, 0:W2], u[:, 0:W2], s[:, 0:W2], AluOpType.min
                    )
                    nc.vector.tensor_tensor(
                        w[:, 0:W2], w[:, 0:W2], s[:, 0:W2], AluOpType.max
                    )
                    # hi0 = min(hi,0) -> s;  hs = max(lo, hi0) -> d
                    nc.vector.tensor_scalar(
                        s[:, 0:W2], w[:, 0:W2], 0.0, None, AluOpType.min
                    )
                    hs = d
                    nc.vector.tensor_tensor(
                        hs[:, 0:W2], u[:, 0:W2], s[:, 0:W2], AluOpType.max
                    )
                    # p = f1 - hs;  q = f1 + hs   (f1[k] = f_sh[k], even)
                    p = wk.tile([128, CMAX + 2], BF16, tag="p")
                    nc.vector.tensor_tensor(
                        p[:, 0:W2], f_sh[:, 0:W2], hs[:, 0:W2],
                        AluOpType.subtract,
                    )
                    q = wk.tile([128, CMAX + 2], BF16, tag="q")
                    nc.vector.tensor_tensor(
                        q[:, 0:W2], f_sh[:, 0:W2], hs[:, 0:W2],
                        AluOpType.add,
                    )
                    # pm = p*mneg -> p;  qm = q*mpos -> q
                    nc.vector.tensor_tensor(
                        p[:, 0:W2], p[:, 0:W2], mneg[:, 0:W2], AluOpType.mult
                    )
                    nc.vector.tensor_tensor(
                        q[:, 0:W2], q[:, 0:W2], mpos[:, 0:W2], AluOpType.mult
                    )
                    # global boundary conditions
                    if c0 == 0:
                        nc.vector.memset(q[:, 0:1], 0.0)
                    if c0 + C == OUT_L:
                        nc.vector.memset(p[:, C + 1:C + 2], 0.0)
                    # fn = pm[1:] + qm[:-1]  (odd src -- degraded mode)
                    fn = u  # u (lo) is dead after hs
                    nc.vector.tensor_tensor(
                        fn[:, 0:C + 1], p[:, 1:C + 2], q[:, 0:C + 1],
                        AluOpType.add,
                    )
                    # out_b = fn[:-1] - fn[1:]  (odd src -- degraded mode)
                    out_b = w  # w (hi) is dead after hi0
                    nc.vector.tensor_tensor(
                        out_b[:, 0:C], fn[:, 0:C], fn[:, 1:C + 1],
                        AluOpType.subtract,
                    )
                    # ACT: final convert bf16 -> fp32
                    out_t = io.tile([128, CMAX], F32, tag="out")
                    nc.scalar.activation(out_t[:, 0:C], out_b[:, 0:C], COPY)
                    nc.sync.dma_start(
                        out[r0:r0 + 128, c0:c0 + C], out_t[:, 0:C]
                    )
                    c0 += C
    _split_multi_waits(nc)
    return nc


def build_module_v6(repeat=1, chunk=2046, wk_bufs=2, pool_ops=(),
                    abs_mode="mixed", masks="ts", f_mixed=False):
    """v6: abs/sign minmod (13 binary ops vs v4's 15), HWDGE-only (works
    inside For_i, so the graded module IS the timed module), Pool-engine
    offload for up to 3 shallow binaries, conversions on ACT.

    Math: d[k] = f[k+1]-f[k]; s = d[k]+d[k+1] = f[k+2]-f[k];
      hs = minmod3(d0, s/4, d1)
         = (sign(d0)+sign(d1)) * min(|d0|/2, |d1|/2, |s|/8)
      p = f1 - hs; q = f1 + hs; pm = (v1<0)*p; qm = (v1>0)*q
      fn = pm[1:] + qm; out = fn[:-1] - fn[1:]

    pool_ops: subset of {"s","pm","qm","q"} to run on the Pool engine.
    abs_on_act: compute |d|/2, |s|/8 on ACT (Abs w/ scale) vs DVE 2-op TS.
    masks: "ts" (DVE is_lt/is_gt @4x) or "act" (sign+relu pair on ACT).
    f_mixed: f = rho_b * v_t(f32) at 1x, dropping the v_b conversion
      (masks then read a pre-shifted ACT-converted v_sh instead).
    """
    import contextlib
    nc = bass.Bass()
    rho = nc.dram_tensor("rho", [ROWS, L], F32, kind="ExternalInput")
    vin = nc.dram_tensor("v", [ROWS, L], F32, kind="ExternalInput")
    out = nc.dram_tensor("out", [ROWS, OUT_L], F32, kind="ExternalOutput")

    SIGN = mybir.ActivationFunctionType.Sign
    RELU = mybir.ActivationFunctionType.Relu
    ABS = mybir.ActivationFunctionType.Abs

    n_chunks = max(1, round(OUT_L / chunk))
    base = (OUT_L // n_chunks) & ~1
    sizes = [base] * (n_chunks - 1) + [OUT_L - base * (n_chunks - 1)]
    assert all(sz % 2 == 0 for sz in sizes) and sum(sizes) == OUT_L, sizes
    CMAX = max(sizes)
    SMAX = CMAX + 4

    def vec_tt(dst, i0, i1, op, eng="dve"):
        if eng == "pool":
            nc.gpsimd.tensor_tensor(dst, i0, i1, op)
        else:
            nc.vector.tensor_tensor(dst, i0, i1, op)

    with TileContext(nc) as tc:
        with (
            tc.tile_pool(name="io", bufs=2) as io,
            tc.tile_pool(name="wk", bufs=wk_bufs) as wk,
            (tc.For_i(0, repeat, 1) if repeat > 1 else contextlib.nullcontext()),
        ):
            for rt in range(RT):
                r0 = rt * 128
                c0 = 0
                for C in sizes:
                    S = C + 4
                    W2 = C + 2
                    W3 = C + 3
                    peng = lambda name: "pool" if name in pool_ops else "dve"

                    # HWDGE fp32 loads
                    rho_t = io.tile([128, SMAX], F32, tag="rho")
                    nc.sync.dma_start(rho_t[:, 0:S], rho[r0:r0 + 128, c0:c0 + S])
                    v_t = io.tile([128, SMAX], F32, tag="v")
                    nc.sync.dma_start(v_t[:, 0:S], vin[r0:r0 + 128, c0:c0 + S])

                    # ACT conversions
                    rho_b = wk.tile([128, SMAX], BF16, tag="rho_b")
                    nc.scalar.activation(rho_b[:, 0:S], rho_t[:, 0:S], COPY)
                    if f_mixed:
                        v_sh = wk.tile([128, SMAX], BF16, tag="v_sh")
                        nc.scalar.activation(v_sh[:, 0:S - 1], v_t[:, 1:S],
                                             COPY)
                    else:
                        v_b = wk.tile([128, SMAX], BF16, tag="v_b")
                        nc.scalar.activation(v_b[:, 0:S], v_t[:, 0:S], COPY)

                    # f = rho * v
                    f = wk.tile([128, SMAX], BF16, tag="f")
                    if f_mixed:
                        nc.vector.tensor_tensor(f[:, 0:S], rho_b[:, 0:S],
                                                v_t[:, 0:S], AluOpType.mult)
                    else:
                        nc.vector.tensor_tensor(f[:, 0:S], rho_b[:, 0:S],
                                                v_b[:, 0:S], AluOpType.mult)
                    # ACT shifted copy f_sh[k] = f[k+1]
                    f_sh = wk.tile([128, SMAX - 1], BF16, tag="f_sh")
                    nc.scalar.activation(f_sh[:, 0:S - 1], f[:, 1:S], COPY)
                    # d[k] = f[k+1]-f[k]  [W3];  s[k] = f[k+2]-f[k]  [W2]
                    d = wk.tile([128, CMAX + 3], BF16, tag="d")
                    nc.vector.tensor_tensor(d[:, 0:W3], f_sh[:, 0:W3],
                                            f[:, 0:W3], AluOpType.subtract)
                    s = wk.tile([128, CMAX + 2], BF16, tag="s")
                    vec_tt(s[:, 0:W2], f[:, 2:C + 4], f[:, 0:W2],
                           AluOpType.subtract, peng("s"))
                    # abs magnitudes.  (Double-scalar TS mult+abs_max does
                    # not compile on this walrus -- verified -- so halving
                    # either rides ACT's input scale or a separate TS mult.)
                    #   act2 : a = |d|/2, as4 = |s|/8 on ACT; hs = t2*mn2
                    #   mixed: a = |d| TS; as4 = |s|/4 ACT; mh = mn2/2 TS
                    #   ts3  : a, as4 via 3 TS ops; mh = mn2/2 TS
                    a = wk.tile([128, CMAX + 3], BF16, tag="a")
                    as4 = wk.tile([128, CMAX + 2], BF16, tag="as4")
                    if abs_mode == "act2":
                        nc.scalar.activation(a[:, 0:W3], d[:, 0:W3], ABS,
                                             scale=0.5)
                        nc.scalar.activation(as4[:, 0:W2], s[:, 0:W2], ABS,
                                             scale=0.125)
                    elif abs_mode == "mixed":
                        nc.vector.tensor_scalar(a[:, 0:W3], d[:, 0:W3],
                                                0.0, None, AluOpType.abs_max)
                        nc.scalar.activation(as4[:, 0:W2], s[:, 0:W2], ABS,
                                             scale=0.25)
                    else:  # ts3
                        nc.vector.tensor_scalar(a[:, 0:W3], d[:, 0:W3],
                                                0.0, None, AluOpType.abs_max)
                        nc.vector.tensor_scalar(as4[:, 0:W2], s[:, 0:W2],
                                                0.0, None, AluOpType.abs_max)
                        nc.vector.tensor_scalar(as4[:, 0:W2], as4[:, 0:W2],
                                                0.25, None, AluOpType.mult)
                    # sg = sign(d) [W3]
                    sg = wk.tile([128, CMAX + 3], BF16, tag="sg")
                    nc.scalar.activation(sg[:, 0:W3], d[:, 0:W3], SIGN)
                    # mn = min(a, a[1:]) [W2];  mn2 = min(mn, as4)
                    mn = wk.tile([128, CMAX + 2], BF16, tag="mn")
                    nc.vector.tensor_tensor(mn[:, 0:W2], a[:, 0:W2],
                                            a[:, 1:W3], AluOpType.min)
                    mn2 = as4  # as4 dead after this
                    nc.vector.tensor_tensor(mn2[:, 0:W2], mn[:, 0:W2],
                                            as4[:, 0:W2], AluOpType.min)
                    # t2 = sg + sg[1:];  hs = t2 * (mn2 or mn2/2)
                    if abs_mode != "act2":
                        # halve once here (act2 pre-halves a/as4 instead)
                        nc.vector.tensor_scalar(mn2[:, 0:W2], mn2[:, 0:W2],
                                                0.5, None, AluOpType.mult)
                    t2 = d  # d dead after a/sg
                    nc.vector.tensor_tensor(t2[:, 0:W2], sg[:, 0:W2],
                                            sg[:, 1:W3], AluOpType.add)
                    hs = mn  # mn dead
                    nc.vector.tensor_tensor(hs[:, 0:W2], t2[:, 0:W2],
                                            mn2[:, 0:W2], AluOpType.mult)
                    # masks from v1 = v[c0+1 : c0+C+3]
                    mneg = wk.tile([128, CMAX + 2], BF16, tag="mneg")
                    mpos = wk.tile([128, CMAX + 2], BF16, tag="mpos")
                    if masks == "ts":
                        vsrc = v_sh[:, 0:W2] if f_mixed else v_b[:, 1:C + 3]
                        nc.vector.tensor_scalar(mneg[:, 0:W2], vsrc, 0.0,
                                                None, AluOpType.is_lt)
                        nc.vector.tensor_scalar(mpos[:, 0:W2], vsrc, 0.0,
                                                None, AluOpType.is_gt)
                    else:
                        sgv = wk.tile([128, CMAX + 2], BF16, tag="sgv")
                        vsrc = v_sh[:, 0:W2] if f_mixed else v_t[:, 1:C + 3]
                        nc.scalar.activation(sgv[:, 0:W2], vsrc, SIGN)
                        nc.scalar.activation(mneg[:, 0:W2], sgv[:, 0:W2],
                                             RELU, scale=-1.0)
                        nc.scalar.activation(mpos[:, 0:W2], sgv[:, 0:W2],
                                             RELU)
                    # p = f_sh - hs;  q = f_sh + hs
                    p = wk.tile([128, CMAX + 2], BF16, tag="p")
                    nc.vector.tensor_tensor(p[:, 0:W2], f_sh[:, 0:W2],
                                            hs[:, 0:W2], AluOpType.subtract)
                    q = wk.tile([128, CMAX + 2], BF16, tag="q")
                    vec_tt(q[:, 0:W2], f_sh[:, 0:W2], hs[:, 0:W2],
                           AluOpType.add, peng("q"))
                    # pm = p*mneg;  qm = q*mpos
                    vec_tt(p[:, 0:W2], p[:, 0:W2], mneg[:, 0:W2],
                           AluOpType.mult, peng("pm"))
                    vec_tt(q[:, 0:W2], q[:, 0:W2], mpos[:, 0:W2],
                           AluOpType.mult, peng("qm"))
                    if c0 == 0:
                        nc.vector.memset(q[:, 0:1], 0.0)
                    if c0 + C == OUT_L:
                        nc.vector.memset(p[:, C + 1:C + 2], 0.0)
                    # fn = pm[1:] + qm;  out_b = fn[:-1] - fn[1:]
                    fn = hs  # hs dead after p/q
                    nc.vector.tensor_tensor(fn[:, 0:C + 1], p[:, 1:C + 2],
                                            q[:, 0:C + 1], AluOpType.add)
                    out_b = p  # p dead
                    nc.vector.tensor_tensor(out_b[:, 0:C], fn[:, 0:C],
                                            fn[:, 1:C + 1],
                                            AluOpType.subtract)
                    # ACT out conversion + HWDGE store
                    out_t = io.tile([128, CMAX], F32, tag="out")
                    nc.scalar.activation(out_t[:, 0:C], out_b[:, 0:C], COPY)
                    nc.sync.dma_start(out[r0:r0 + 128, c0:c0 + C],
                                      out_t[:, 0:C])
                    c0 += C
    _split_multi_waits(nc)
    return nc


def build_module_v7(repeat=1, chunk=2730, wk_bufs=2, masks="ts",
                    relu_act=False, scale_act=False, rt_inner=False,
                    fuse_conv=False, f_f32=False, io_bufs=2,
                    masks_early=False, store_q="sp", taper=0, pad=0,
                    pqfuse=False, v_act_q=False, edge_split=False,
                    sgv_psum=False):
    """v7: minimal-sync DVE-centric variant (HWDGE-only, For_i-safe).

    Insights driving it (all HW-measured this session):
      * odd bf16 source offsets cost only ~+24% on TT (not 2x) -> shifted
        VIEWS beat materialized shifted copies; d1 = d[1:] is free.
      * clamp-form minmod saves a TT vs the min/max ladder:
          hs' = min(max(s'/4, min(m2',0)), relu(m1'))   [negated slopes]
        with m1' = min(d', d'[1:]), m2' = max(d', d'[1:]).
      * TS (tensor_scalar) mult/min/max/is_lt/is_gt run ~4x -> masks,
        0.25-scale, clamp bounds all stay on DVE, eliminating the ACT
        round-trips that dominated measured-vs-engine-busy gap.
      * cross-engine hops per chunk: DMA->ACT(conv x2)->DVE(whole
        chain)->ACT(out conv)->DMA.

    Negation bookkeeping (d' = -d0 = f[k]-f[k+1]):
      hs' = -hs;  p = f1 + hs';  q = f1 - hs'.
    """
    import contextlib
    nc = bass.Bass()
    rho = nc.dram_tensor("rho", [ROWS, L], F32, kind="ExternalInput")
    vin = nc.dram_tensor("v", [ROWS, L], F32, kind="ExternalInput")
    out = nc.dram_tensor("out", [ROWS, OUT_L], F32, kind="ExternalOutput")

    if taper:
        # small first/last chunks shrink the pipeline ramp (first DVE op
        # waits only for a small load+convert) and the drain tail.
        w0 = taper & ~1
        inner = OUT_L - 2 * w0
        n_in = max(1, round(inner / chunk))
        base = (inner // n_in) & ~1
        sizes = ([w0] + [base] * (n_in - 1)
                 + [inner - base * (n_in - 1)] + [w0])
    else:
        n_chunks = max(1, round(OUT_L / chunk))
        base = (OUT_L // n_chunks) & ~1
        sizes = [base] * (n_chunks - 1) + [OUT_L - base * (n_chunks - 1)]
    assert all(sz % 2 == 0 for sz in sizes) and sum(sizes) == OUT_L, sizes
    # pad widens every tile allocation without changing any computation
    # slice -- shifts the SBUF layout (bank-conflict lottery knob).
    CMAX = max(sizes) + pad
    SMAX = CMAX + 4

    with TileContext(nc) as tc:
        with (
            tc.tile_pool(name="io", bufs=io_bufs) as io,
            tc.tile_pool(name="wk", bufs=wk_bufs) as wk,
            (tc.psum_pool(name="ps", bufs=2) if sgv_psum
             else contextlib.nullcontext()) as ps,
            (tc.For_i(0, repeat, 1) if repeat > 1 else contextlib.nullcontext()),
        ):
            # rt_inner interleaves the four row-tiles at each chunk column,
            # giving the scheduler 4 independent chains to fill bubbles.
            units = [(rt, ci) for rt in range(RT) for ci in range(len(sizes))]
            if rt_inner:
                units = [(rt, ci) for ci in range(len(sizes))
                         for rt in range(RT)]
            offs = [sum(sizes[:i]) for i in range(len(sizes))]
            for rt, ci in units:
                    r0 = rt * 128
                    c0 = offs[ci]
                    C = sizes[ci]
                    S = C + 4
                    W2 = C + 2
                    W3 = C + 3
                    if fuse_conv:
                        # one f32 staging tile holds rho|v gap-free ->
                        # a single wide ACT conversion covers both.
                        rv_t = io.tile([128, 2 * SMAX], F32, tag="rv")
                        nc.sync.dma_start(rv_t[:, 0:S],
                                          rho[r0:r0 + 128, c0:c0 + S])
                        # v of the FIRST unit rides the ACT DMA queue so the
                        # two ramp-critical loads run concurrently (later
                        # units would queue behind ACT compute -- measured
                        # worse -- so only unit 0 is redirected).
                        first = (rt, ci) == units[0]
                        v_dma_eng = (nc.scalar if (v_act_q and first)
                                     else nc.sync)
                        if edge_split and first:
                            # ramp: halve unit 0's v load so its low-half
                            # conversion overlaps the high-half DMA.
                            H2 = (S // 2) & ~1
                            v_dma_eng.dma_start(
                                rv_t[:, S:S + H2],
                                vin[r0:r0 + 128, c0:c0 + H2])
                            v_dma_eng.dma_start(
                                rv_t[:, S + H2:2 * S],
                                vin[r0:r0 + 128, c0 + H2:c0 + S])
                        else:
                            v_dma_eng.dma_start(rv_t[:, S:2 * S],
                                                vin[r0:r0 + 128, c0:c0 + S])
                        rv_b = wk.tile([128, 2 * SMAX], BF16, tag="rv_b")
                        if edge_split and (rt, ci) == units[0]:
                            # ramp: split the first unit's conversion so the
                            # rho half runs while the v DMA is still in
                            # flight (loads are serial -- HBM-bound), and
                            # the v low half converts under the v high DMA.
                            H2 = (S // 2) & ~1
                            nc.scalar.activation(rv_b[:, 0:S],
                                                 rv_t[:, 0:S], COPY)
                            nc.scalar.activation(rv_b[:, S:S + H2],
                                                 rv_t[:, S:S + H2], COPY)
                            nc.scalar.activation(rv_b[:, S + H2:2 * S],
                                                 rv_t[:, S + H2:2 * S],
                                                 COPY)
                        else:
                            nc.scalar.activation(rv_b[:, 0:2 * S],
                                                 rv_t[:, 0:2 * S], COPY)
                        rho_b = rv_b[:, 0:SMAX]
                        v_b = rv_b[:, S:S + SMAX]
                        v_t = rv_t[:, S:S + SMAX]
                    elif f_f32:
                        # no conversions: f multiplies the f32 staging
                        # tiles directly (1x DVE) -- frees ~27% of ACT.
                        # masks must be "act"/"hybrid" (no v_b): sgv reads
                        # v_t.
                        rho_t = io.tile([128, SMAX], F32, tag="rho")
                        nc.sync.dma_start(rho_t[:, 0:S],
                                          rho[r0:r0 + 128, c0:c0 + S])
                        v_t = io.tile([128, SMAX], F32, tag="v")
                        nc.sync.dma_start(v_t[:, 0:S],
                                          vin[r0:r0 + 128, c0:c0 + S])
                        rho_b = rho_t
                        v_b = v_t
                    else:
                        # HWDGE fp32 loads
                        rho_t = io.tile([128, SMAX], F32, tag="rho")
                        nc.sync.dma_start(rho_t[:, 0:S],
                                          rho[r0:r0 + 128, c0:c0 + S])
                        v_t = io.tile([128, SMAX], F32, tag="v")
                        nc.sync.dma_start(v_t[:, 0:S],
                                          vin[r0:r0 + 128, c0:c0 + S])
                        # ACT conversions (only engine hop before the chain)
                        rho_b = wk.tile([128, SMAX], BF16, tag="rho_b")
                        nc.scalar.activation(rho_b[:, 0:S], rho_t[:, 0:S],
                                             COPY)
                        v_b = wk.tile([128, SMAX], BF16, tag="v_b")
                        nc.scalar.activation(v_b[:, 0:S], v_t[:, 0:S], COPY)
                    # masks from v1 = v_b[1:C+3] / v_t[1:C+3].  With pqfuse,
                    # mneg|mpos live adjacently in one tile so the two mask
                    # multiplies later collapse into one double-width TT.
                    if pqfuse:
                        mm = wk.tile([128, 2 * (CMAX + 2)], BF16, tag="mm")
                        mneg = mm[:, 0:CMAX + 2]
                        mpos = mm[:, W2:W2 + CMAX + 2]
                    else:
                        mneg = wk.tile([128, CMAX + 2], BF16, tag="mneg")
                        mpos = wk.tile([128, CMAX + 2], BF16, tag="mpos")

                    def emit_masks():
                        if masks == "act":
                            SIGN = mybir.ActivationFunctionType.Sign
                            RELU = mybir.ActivationFunctionType.Relu
                            # sgv in PSUM keeps its 3 ACT accesses off the
                            # SBUF ports (probe for SBUF port contention
                            # with DVE); PSUM is also faster for ACT.
                            sgv_pool = ps if sgv_psum else wk
                            sgv = sgv_pool.tile([128, CMAX + 2], BF16,
                                                tag="sgv")
                            nc.scalar.activation(sgv[:, 0:W2],
                                                 v_t[:, 1:C + 3], SIGN)
                            nc.scalar.activation(mneg[:, 0:W2],
                                                 sgv[:, 0:W2], RELU,
                                                 scale=-1.0)
                            nc.scalar.activation(mpos[:, 0:W2],
                                                 sgv[:, 0:W2], RELU)
                        elif masks == "hybrid":
                            SIGN = mybir.ActivationFunctionType.Sign
                            RELU = mybir.ActivationFunctionType.Relu
                            sgv = wk.tile([128, CMAX + 2], BF16, tag="sgv")
                            nc.scalar.activation(sgv[:, 0:W2],
                                                 v_t[:, 1:C + 3], SIGN)
                            nc.scalar.activation(mpos[:, 0:W2],
                                                 sgv[:, 0:W2], RELU)
                            nc.vector.tensor_scalar(mneg[:, 0:W2],
                                                    v_b[:, 1:C + 3],
                                                    0.0, None,
                                                    AluOpType.is_lt)
                        else:
                            nc.vector.tensor_scalar(mneg[:, 0:W2],
                                                    v_b[:, 1:C + 3],
                                                    0.0, None,
                                                    AluOpType.is_lt)
                            nc.vector.tensor_scalar(mpos[:, 0:W2],
                                                    v_b[:, 1:C + 3],
                                                    0.0, None,
                                                    AluOpType.is_gt)

                    if masks_early:
                        emit_masks()
                    # f = rho*v
                    f = wk.tile([128, SMAX], BF16, tag="f")
                    nc.vector.tensor_tensor(f[:, 0:S], rho_b[:, 0:S],
                                            v_b[:, 0:S], AluOpType.mult)
                    # d'[k] = f[k] - f[k+1]  [W3]; d1' is the view d'[1:]
                    d = wk.tile([128, CMAX + 3], BF16, tag="d")
                    nc.vector.tensor_tensor(d[:, 0:W3], f[:, 0:W3],
                                            f[:, 1:S], AluOpType.subtract)
                    # s' = f - f[2:]  [W2]; b' = s'/4 in place
                    s = wk.tile([128, CMAX + 2], BF16, tag="s")
                    nc.vector.tensor_tensor(s[:, 0:W2], f[:, 0:W2],
                                            f[:, 2:C + 4], AluOpType.subtract)
                    if scale_act:
                        nc.scalar.activation(s[:, 0:W2], s[:, 0:W2], COPY,
                                             scale=0.25)
                    else:
                        nc.vector.tensor_scalar(s[:, 0:W2], s[:, 0:W2], 0.25,
                                                None, AluOpType.mult)
                    # m1' = min(d', d'[1:]);  m2' = max(d', d'[1:])
                    m1 = wk.tile([128, CMAX + 2], BF16, tag="m1")
                    nc.vector.tensor_tensor(m1[:, 0:W2], d[:, 0:W2],
                                            d[:, 1:W3], AluOpType.min)
                    m2 = wk.tile([128, CMAX + 2], BF16, tag="m2")
                    nc.vector.tensor_tensor(m2[:, 0:W2], d[:, 0:W2],
                                            d[:, 1:W3], AluOpType.max)
                    # A = min(m2', 0) in place;  R = relu(m1') in place
                    nc.vector.tensor_scalar(m2[:, 0:W2], m2[:, 0:W2], 0.0,
                                            None, AluOpType.min)
                    if relu_act:
                        nc.scalar.activation(
                            m1[:, 0:W2], m1[:, 0:W2],
                            mybir.ActivationFunctionType.Relu)
                    else:
                        nc.vector.tensor_scalar(m1[:, 0:W2], m1[:, 0:W2],
                                                0.0, None, AluOpType.max)
                    # t1 = max(b', A) -> s;  hs' = min(t1, R) -> m1
                    nc.vector.tensor_tensor(s[:, 0:W2], s[:, 0:W2],
                                            m2[:, 0:W2], AluOpType.max)
                    hs = m1
                    nc.vector.tensor_tensor(hs[:, 0:W2], s[:, 0:W2],
                                            m1[:, 0:W2], AluOpType.min)
                    # p = f[1:] + hs';  q = f[1:] - hs'
                    if pqfuse:
                        pq = wk.tile([128, 2 * (CMAX + 2)], BF16, tag="pq")
                        p = pq[:, 0:CMAX + 2]
                        q = pq[:, W2:W2 + CMAX + 2]
                    else:
                        p = wk.tile([128, CMAX + 2], BF16, tag="p")
                        q = wk.tile([128, CMAX + 2], BF16, tag="q")
                    nc.vector.tensor_tensor(p[:, 0:W2], f[:, 1:C + 3],
                                            hs[:, 0:W2], AluOpType.add)
                    nc.vector.tensor_tensor(q[:, 0:W2], f[:, 1:C + 3],
                                            hs[:, 0:W2], AluOpType.subtract)
                    if not masks_early:
                        emit_masks()
                    # pm = p*mneg -> p;  qm = q*mpos -> q.  With pqfuse the
                    # two multiplies are one double-width TT over pq|mm.
                    if pqfuse:
                        nc.vector.tensor_tensor(pq[:, 0:2 * W2],
                                                pq[:, 0:2 * W2],
                                                mm[:, 0:2 * W2],
                                                AluOpType.mult)
                    else:
                        nc.vector.tensor_tensor(p[:, 0:W2], p[:, 0:W2],
                                                mneg[:, 0:W2],
                                                AluOpType.mult)
                        nc.vector.tensor_tensor(q[:, 0:W2], q[:, 0:W2],
                                                mpos[:, 0:W2],
                                                AluOpType.mult)
                    if c0 == 0:
                        nc.vector.memset(q[:, 0:1], 0.0)
                    if c0 + C == OUT_L:
                        nc.vector.memset(p[:, C + 1:C + 2], 0.0)
                    # fn = pm[1:] + qm -> d (dead);  out_b = fn - fn[1:] -> s
                    fn = d
                    nc.vector.tensor_tensor(fn[:, 0:C + 1], p[:, 1:C + 2],
                                            q[:, 0:C + 1], AluOpType.add)
                    out_b = s
                    nc.vector.tensor_tensor(out_b[:, 0:C], fn[:, 0:C],
                                            fn[:, 1:C + 1],
                                            AluOpType.subtract)
                    # ACT out conversion + HWDGE store
                    out_t = io.tile([128, CMAX], F32, tag="out")
                    store_eng = nc.vector if store_q == "dve" else nc.sync
                    if edge_split and (rt, ci) == units[-1]:
                        # tail: halve the last unit's convert+store so the
                        # first store overlaps the second conversion.
                        H = (C // 2) & ~1
                        nc.scalar.activation(out_t[:, 0:H], out_b[:, 0:H],
                                             COPY)
                        store_eng.dma_start(out[r0:r0 + 128, c0:c0 + H],
                                            out_t[:, 0:H])
                        nc.scalar.activation(out_t[:, H:C], out_b[:, H:C],
                                             COPY)
                        store_eng.dma_start(
                            out[r0:r0 + 128, c0 + H:c0 + C], out_t[:, H:C])
                    else:
                        nc.scalar.activation(out_t[:, 0:C], out_b[:, 0:C],
                                             COPY)
                        store_eng.dma_start(out[r0:r0 + 128, c0:c0 + C],
                                            out_t[:, 0:C])
                    c0 += C
    _split_multi_waits(nc)
    return nc


# Shipped configuration: v7 (see build_module_v7 docstring).  The same
# module is used for the graded kernel() call and (with repeat>1) for the
# For_i device-time measurement in test.py -- HWDGE-only, so it compiles
# inside the hardware loop.
V7_CFG = dict(chunk=2046, masks="act", relu_act=True, scale_act=True,
              fuse_conv=True, rt_inner=True, pqfuse=True, edge_split=True)

_NC_CACHE = None


def _get_nc():
    global _NC_CACHE
    if _NC_CACHE is None:
        _NC_CACHE = build_module_v7(**V7_CFG)
    return _NC_CACHE


def kernel(rho, v, axis=2, retain_padding=0, **_kw):
    rho = np.ascontiguousarray(np.asarray(rho, dtype=np.float32))
    v = np.ascontiguousarray(np.asarray(v, dtype=np.float32))
    assert rho.shape == (B, M, L) and v.shape == (B, M, L)

    nc = _get_nc()
    in_maps = [
        {
            "rho": rho[c * BP:(c + 1) * BP].reshape(ROWS, L),
            "v": v[c * BP:(c + 1) * BP].reshape(ROWS, L),
        }
        for c in range(NCORES)
    ]
    last_err = None
    for _attempt in range(3):
        try:
            res = run_bass_kernel_spmd(
                nc, in_maps, core_ids=list(range(NCORES))
            )
            break
        except Exception as e:  # rare transient NRT device errors
            last_err = e
            import time as _time
            _time.sleep(5)
    else:
        raise last_err
    outs = [r["out"].reshape(BP, M, OUT_L) for r in res.results]
    return np.concatenate(outs, axis=0)


def build_module_v5(repeat=1, chunk=1636, wk_bufs=2):
    """v5: whole-input-resident variant.

    Phase 1 (outside the repeat loop): SWDGE cast-loads of ALL of rho/v
    into persistent SBUF bf16 buffers (16.8 MB of 28 MB SBUF).
    Phase 2 (loopable): compute + ACT out-conversion + HWDGE fp32 store
    -- contains no SWDGE, so it compiles inside For_i for benchmarking.
    """
    import contextlib
    nc = bass.Bass()
    rho = nc.dram_tensor("rho", [ROWS, L], F32, kind="ExternalInput")
    vin = nc.dram_tensor("v", [ROWS, L], F32, kind="ExternalInput")
    out = nc.dram_tensor("out", [ROWS, OUT_L], F32, kind="ExternalOutput")

    SIGN = mybir.ActivationFunctionType.Sign
    RELU = mybir.ActivationFunctionType.Relu

    n_chunks = max(1, round(OUT_L / chunk))
    base = (OUT_L // n_chunks) & ~1
    sizes = [base] * (n_chunks - 1) + [OUT_L - base * (n_chunks - 1)]
    assert all(sz % 2 == 0 for sz in sizes) and sum(sizes) == OUT_L, sizes
    CMAX = max(sizes)

    with TileContext(nc) as tc:
        with (
            tc.tile_pool(name="inp", bufs=1) as inp,
            tc.tile_pool(name="io", bufs=2) as io,
            tc.tile_pool(name="wk", bufs=wk_bufs) as wk,
        ):
            rho_sb = inp.tile([128, RT, L], BF16, tag="rho_sb")
            v_sb = inp.tile([128, RT, L], BF16, tag="v_sb")
            for rt in range(RT):
                r0 = rt * 128
                nc.gpsimd.dma_start(rho_sb[:, rt, :], rho[r0:r0 + 128, :])
                nc.gpsimd.dma_start(v_sb[:, rt, :], vin[r0:r0 + 128, :])
            with (tc.For_i(0, repeat, 1) if repeat > 1
                  else contextlib.nullcontext()):
                for rt in range(RT):
                    r0 = rt * 128
                    c0 = 0
                    for C in sizes:
                        S = C + 4
                        W2 = C + 2
                        rv = rho_sb[:, rt, c0:c0 + S]
                        vv = v_sb[:, rt, c0:c0 + S]
                        # ACT masks: sgn = sign(v1); mneg = relu(-sgn);
                        # mpos = relu(sgn) in-place.
                        sgn = wk.tile([128, CMAX + 2], BF16, tag="sgn")
                        nc.scalar.activation(
                            sgn[:, 0:W2], v_sb[:, rt, c0 + 1:c0 + C + 3],
                            SIGN,
                        )
                        mneg = wk.tile([128, CMAX + 2], BF16, tag="mneg")
                        nc.scalar.activation(
                            mneg[:, 0:W2], sgn[:, 0:W2], RELU, scale=-1.0
                        )
                        mpos = sgn
                        nc.scalar.activation(
                            mpos[:, 0:W2], sgn[:, 0:W2], RELU
                        )
                        # DVE: f = rho*v
                        f = wk.tile([128, CMAX + 4], BF16, tag="f")
                        nc.vector.tensor_tensor(
                            f[:, 0:S], rv, vv, AluOpType.mult
                        )
                        # s' = f[i]-f[i+2]
                        s = wk.tile([128, CMAX + 2], BF16, tag="s")
                        nc.vector.tensor_tensor(
                            s[:, 0:W2], f[:, 0:W2], f[:, 2:C + 4],
                            AluOpType.subtract,
                        )
                        # ACT: f_sh[k] = f[k+1];  s4' = 0.25*s'
                        f_sh = wk.tile([128, CMAX + 3], BF16, tag="f_sh")
                        nc.scalar.activation(
                            f_sh[:, 0:S - 1], f[:, 1:S], COPY
                        )
                        nc.scalar.activation(
                            s[:, 0:W2], s[:, 0:W2], COPY, scale=0.25
                        )
                        # d' = f - f_sh;  d1' = f_sh - f[2:]
                        d = wk.tile([128, CMAX + 2], BF16, tag="d")
                        nc.vector.tensor_tensor(
                            d[:, 0:W2], f[:, 0:W2], f_sh[:, 0:W2],
                            AluOpType.subtract,
                        )
                        d1 = wk.tile([128, CMAX + 2], BF16, tag="d1")
                        nc.vector.tensor_tensor(
                            d1[:, 0:W2], f_sh[:, 0:W2], f[:, 2:C + 4],
                            AluOpType.subtract,
                        )
                        # u' = max(d',d1') -> nlo;  w' = min(d',d1') -> nhi
                        u = wk.tile([128, CMAX + 2], BF16, tag="u")
                        nc.vector.tensor_tensor(
                            u[:, 0:W2], d[:, 0:W2], d1[:, 0:W2],
                            AluOpType.max,
                        )
                        w = wk.tile([128, CMAX + 2], BF16, tag="w")
                        nc.vector.tensor_tensor(
                            w[:, 0:W2], d[:, 0:W2], d1[:, 0:W2],
                            AluOpType.min,
                        )
                        nc.vector.tensor_tensor(
                            u[:, 0:W2], u[:, 0:W2], s[:, 0:W2],
                            AluOpType.max,
                        )
                        nc.vector.tensor_tensor(
                            w[:, 0:W2], w[:, 0:W2], s[:, 0:W2],
                            AluOpType.min,
                        )
                        # ACT: nhi0 = relu(nhi) -> s
                        nc.scalar.activation(s[:, 0:W2], w[:, 0:W2], RELU)
                        # hs' = min(nlo, nhi0) -> d
                        hs = d
                        nc.vector.tensor_tensor(
                            hs[:, 0:W2], u[:, 0:W2], s[:, 0:W2],
                            AluOpType.min,
                        )
                        # p = f_sh + hs';  q = f_sh - hs'
                        p = wk.tile([128, CMAX + 2], BF16, tag="p")
                        nc.vector.tensor_tensor(
                            p[:, 0:W2], f_sh[:, 0:W2], hs[:, 0:W2],
                            AluOpType.add,
                        )
                        q = wk.tile([128, CMAX + 2], BF16, tag="q")
                        nc.vector.tensor_tensor(
                            q[:, 0:W2], f_sh[:, 0:W2], hs[:, 0:W2],
                            AluOpType.subtract,
                        )
                        # pm, qm
                        nc.vector.tensor_tensor(
                            p[:, 0:W2], p[:, 0:W2], mneg[:, 0:W2],
                            AluOpType.mult,
                        )
                        nc.vector.tensor_tensor(
                            q[:, 0:W2], q[:, 0:W2], mpos[:, 0:W2],
                            AluOpType.mult,
                        )
                        if c0 == 0:
                            nc.vector.memset(q[:, 0:1], 0.0)
                        if c0 + C == OUT_L:
                            nc.vector.memset(p[:, C + 1:C + 2], 0.0)
                        # fn = pm[1:] + qm[:-1];  out_b = fn[:-1]-fn[1:]
                        fn = u
                        nc.vector.tensor_tensor(
                            fn[:, 0:C + 1], p[:, 1:C + 2], q[:, 0:C + 1],
                            AluOpType.add,
                        )
                        out_b = w
                        nc.vector.tensor_tensor(
                            out_b[:, 0:C], fn[:, 0:C], fn[:, 1:C + 1],
                            AluOpType.subtract,
                        )
                        # ACT: convert bf16 -> fp32; HWDGE store (loop-safe)
                        out_t = io.tile([128, CMAX], F32, tag="out")
                        nc.scalar.activation(
                            out_t[:, 0:C], out_b[:, 0:C], COPY
                        )
                        nc.sync.dma_start(
                            out[r0:r0 + 128, c0:c0 + C], out_t[:, 0:C]
                        )
                        c0 += C
    _split_multi_waits(nc)
    return nc

